# revision 1
# baseline (speedup 1.0000x reference)
"""CapsNet dynamic-routing layer on 8 Trainium2 NeuronCores (Bass/Tile).

reference math (per batch element b):
  u_hat[b,i,o,j] = sum_d W[i,o,j,d] * u[b,i,d]        (never materialized)
  bl = 0; for r in 0..2:
    c = softmax_o(bl); s[b,o,j] = sum_i c*u_hat; v = squash(s)
    if r < 2: bl += sum_j u_hat*v
  return v  [B, 10, 16]

Distribution: pure data parallel, batch 512 -> 64 per core x 8 cores;
weights replicated.  Per-core: b=64, i=1152=9*128, o=10, j=16, d=8.

Key layout trick: o is mapped to PE column/row strips as g=o%4 (strip)
and sl=o//4 (slot), consistently across the s-matmuls (col-tiled),
squash (strip-local), the agreement matmuls (row-tiled), and the
output transposes - so no partition-moving shuffles are ever needed.
The softmax splits o as h=o//5 across partition halves (paired with
the agreement-pass PSUM packing) and o5=o%5 along free.
"""
import sys

sys.path.insert(0, "/opt/trn_rl_repo")

import numpy as np
import ml_dtypes
from contextlib import ExitStack

from concourse import bacc, mybir, hw_specs
from concourse.tile import TileContext
from concourse.bass_utils import run_bass_kernel_spmd

BF16 = mybir.dt.bfloat16
F32 = mybir.dt.float32
AX = mybir.AxisListType
ALU = mybir.AluOpType
ACTF = mybir.ActivationFunctionType
bfnp = ml_dtypes.bfloat16

B = 64
I = 1152
T = 9
O = 10
J = 16
D = 8
EPS = 1e-06
N_CORES = 8
KFLAT = D * I          # 9216 (d-major flat)
NCH = KFLAT // 512     # 18

_cache = {}

# Route every activation through the one table set that has exp+ln+copy,
# so the ACT engine never reloads tables mid-kernel.  Entry order (and
# hence act_func_set_id indices) is preserved.
_KEEP_SET = "natural_log_exp_and_others"


def _patched_tables(arch):
    full = {k: set(v) for k, v in hw_specs.get_activation_tables(arch).items()}
    keep = full[_KEEP_SET]
    return {k: (v if k == _KEEP_SET else v - keep) for k, v in full.items()}


import os
if os.environ.get('ACT_PATCH', '1') == '1':
    bacc.get_activation_tables = _patched_tables


def build_nc():
    nc = bacc.Bacc()
    ws_d = nc.dram_tensor("ws", [128, T, D, O, J], BF16, kind="ExternalInput")
    wb_d = nc.dram_tensor("wb", [128, 3 * KFLAT], BF16, kind="ExternalInput")
    ui_d = nc.dram_tensor("ui", [128, T, D, B], BF16, kind="ExternalInput")
    ur_d = nc.dram_tensor("ur", [128, KFLAT], BF16, kind="ExternalInput")
    cid2_d = nc.dram_tensor("cid2", [128, 64], F32, kind="ExternalInput")
    cid16s_d = nc.dram_tensor("cid16s", [128, 16], F32, kind="ExternalInput")
    cones128_d = nc.dram_tensor("cones128", [128, 1], F32, kind="ExternalInput")
    cones1_d = nc.dram_tensor("cones1", [1, 16], F32, kind="ExternalInput")
    vout_d = nc.dram_tensor("vout", [B, O, J], F32, kind="ExternalOutput")

    with TileContext(nc) as tc, ExitStack() as ctx:
        static = ctx.enter_context(tc.tile_pool(name="static", bufs=1))
        work = ctx.enter_context(tc.tile_pool(name="work", bufs=1))
        cupool = ctx.enter_context(tc.tile_pool(name="cup", bufs=2))
        psA = ctx.enter_context(tc.tile_pool(name="psA", bufs=1, space="PSUM"))
        psB = ctx.enter_context(tc.tile_pool(name="psB", bufs=3, space="PSUM"))
        psC = ctx.enter_context(tc.tile_pool(name="psC", bufs=2, space="PSUM"))
        psD = ctx.enter_context(tc.tile_pool(name="psD", bufs=2, space="PSUM"))

        ws = static.tile([128, T, D, O, J], BF16, name="ws")
        wb = static.tile([128, 3 * KFLAT], BF16, name="wb")
        ui = static.tile([128, T, D, B], BF16, name="ui")
        ur = static.tile([128, KFLAT], BF16, name="ur")
        cid2 = static.tile([128, 64], F32, name="cid2")
        cid16s = static.tile([128, 16], F32, name="cid16s")
        cones128 = static.tile([128, 1], F32, name="cones128")
        cones1 = static.tile([1, 16], F32, name="cones1")
        nc.sync.dma_start(out=ws, in_=ws_d[:, :, :, :, :])
        nc.sync.dma_start(out=wb, in_=wb_d[:, :])
        nc.sync.dma_start(out=ui, in_=ui_d[:, :, :, :])
        nc.sync.dma_start(out=ur, in_=ur_d[:, :])
        nc.sync.dma_start(out=cid2, in_=cid2_d[:, :])
        nc.sync.dma_start(out=cid16s, in_=cid16s_d[:, :])
        nc.sync.dma_start(out=cones128, in_=cones128_d[:, :])
        nc.sync.dma_start(out=cones1, in_=cones1_d[:, :])

        bl = work.tile([128, 5, I], F32, name="bl")
        c_t = work.tile([128, T, O, B], BF16, name="c_t")
        zh = work.tile([128, I], F32, name="zh")
        rz = work.tile([128, I], F32, name="rz")
        scratch = work.tile([128, KFLAT], BF16, name="scratch")
        ug = scratch
        e = scratch[:, 0 : 5 * I].rearrange("p (o i) -> p o i", o=5)
        s_sb = work.tile([128, 3, B], F32, name="s_sb")
        s2 = work.tile([128, 3, B], F32, name="s2")
        v_sb = work.tile([128, 3, B], F32, name="v_sb")
        v_st = work.tile([128, 3, B], BF16, name="v_st")
        sq_sb = work.tile([1, 4, 3, B], F32, name="sq_sb")
        t1p = work.tile([1, 768], F32, name="t1p")
        t2p = work.tile([1, 768], F32, name="t2p")
        den = work.tile([1, 768], F32, name="den")
        rec = work.tile([1, 768], F32, name="rec")
        v_t = work.tile([64, O, J], F32, name="v_t")
        eps1 = work.tile([1, 1], F32, name="eps1")
        nc.vector.memset(eps1, EPS)
        nc.vector.memset(s_sb.rearrange("p s b -> p (s b)"), 0.0)
        nc.vector.memset(sq_sb.rearrange("p g s b -> p (g s b)"), 0.0)

        PAIRS = [(0, 1), (2, 3), (4, 5), (6, 7), (8, 9)]

        def m1_pair(pair, rhs_of, scale):
            """col-tiled s matmuls for an o-pair -> s_sb strips."""
            ps = psA.tile([128, B], F32, name="m1ps", tag="m1ps")
            for t in range(T):
                for d in range(D):
                    for o in pair:
                        g = o % 4
                        nc.tensor.matmul(
                            ps[32 * g : 32 * g + 16, :],
                            ws[:, t, d, o, :],
                            rhs_of(o)[:, t, d, :],
                            start=(t == 0 and d == 0),
                            stop=(t == T - 1 and d == D - 1),
                            tile_position=(0, 32 * g),
                        )
            for o in pair:
                g, slot = o % 4, o // 4
                nc.scalar.mul(s_sb[32 * g : 32 * g + 16, slot, :],
                              ps[32 * g : 32 * g + 16, :], scale)

        def squash():
            """v_sb = squash(s_sb) with j on partitions (strip-local)."""
            sf = s_sb.rearrange("p s b -> p (s b)")
            s2f = s2.rearrange("p s b -> p (s b)")
            nc.vector.tensor_tensor(s2f, sf, sf, op=ALU.mult)
            for g in range(4):
                nsl = 3 if g < 2 else 2
                sqg = psD.tile([1, 3 * B], F32, name="sqg", tag="sqps")
                nc.tensor.matmul(
                    sqg[:, 0 : nsl * B],
                    cones128[32 * g : 32 * g + 16, :],
                    s2[32 * g : 32 * g + 16, 0:nsl, :],
                    start=True, stop=True,
                    tile_position=(32 * g, 0),
                )
                # scatter group's o-slices (o = g + 4*sl) into sq_sb
                nc.vector.tensor_copy(
                    sq_sb[:, g, 0:nsl, :],
                    sqg[:, 0 : nsl * B].rearrange("p (s b) -> p s b", s=nsl),
                )
            # o-major view of sq_sb: o = g + 4*sl  ->  dims (sl, g, b)
            sqv = sq_sb.transpose([0, 2, 1, 3])
            def _v(ap):
                return ap.rearrange("p (s g b) -> p s g b", s=3, g=4)
            nc.scalar.activation(_v(t1p), sqv, ACTF.Ln, bias=eps1)
            nc.scalar.activation(t2p, t1p, ACTF.Exp, scale=0.5)
            nc.vector.tensor_scalar_add(_v(den), sqv, 1.0)
            nc.vector.tensor_tensor(den, den, t2p, op=ALU.mult)
            nc.vector.reciprocal_approx_accurate(rec, den, t1p)
            nc.vector.tensor_tensor(_v(den), sqv, _v(rec), op=ALU.mult)
            mrep = psC.tile([128, 3 * B], F32, name="mrep", tag="miscps")
            nc.vector.memset(mrep, 0.0)
            for o in range(O):
                g, sl = o % 4, o // 4
                nc.tensor.matmul(
                    mrep[32 * g : 32 * g + 16, 64 * sl : 64 * (sl + 1)],
                    cones1,
                    den[:, 64 * o : 64 * (o + 1)],
                    start=True, stop=True,
                    tile_position=(0, 32 * g),
                )
            vf = v_sb.rearrange("p s b -> p (s b)")
            nc.vector.tensor_tensor(vf, sf, mrep, op=ALU.mult)

        def m2_b2(it):
            """bl (+)= sum_j u_hat * v   (g = W.T@v row-tiled; u*g; d-tree)."""
            nc.vector.tensor_copy(v_st.rearrange("p s b -> p (s b)"),
                                  v_sb.rearrange("p s b -> p (s b)"))
            for p in range(5):
                for n in range(NCH // 2):
                    for nn in (n, n + 9):
                        ps = psB.tile([128, 512], F32, name="m2ps", tag="m2ps")
                        for half, o in ((0, p), (1, p + 5)):
                            g, sl = o % 4, o // 4
                            nc.tensor.matmul(
                                ps[64 * half : 64 * half + 64, :],
                                v_st[32 * g : 32 * g + 16, sl, :],
                                wb[32 * g : 32 * g + 16,
                                   sl * KFLAT + 512 * nn : sl * KFLAT + 512 * (nn + 1)],
                                start=True, stop=True,
                                tile_position=(32 * g, 64 * half),
                            )
                        nc.scalar.copy(ug[:, 512 * nn : 512 * (nn + 1)], ps)
                    for nn in (n, n + 9):
                        sl = slice(512 * nn, 512 * (nn + 1))
                        nc.vector.tensor_tensor(ug[:, sl], ug[:, sl], ur[:, sl],
                                                op=ALU.mult)
                    sl = slice(512 * (n + 9), 512 * (n + 10))
                    nc.vector.tensor_tensor(
                        ug[:, sl], ug[:, 512 * n : 512 * (n + 1)],
                        ug[:, sl], op=ALU.add)
                h, q = KFLAT // 2, KFLAT // 4
                # l1 lives in [h:KFLAT); fold its halves into [h:h+q)
                nc.vector.tensor_tensor(ug[:, h : h + q], ug[:, h : h + q],
                                        ug[:, h + q : KFLAT], op=ALU.add)
                l3 = ug[:, h + q : h + q + 2 * I].bitcast(F32)
                nc.vector.tensor_tensor(l3, ug[:, h : h + I],
                                        ug[:, h + I : h + 2 * I], op=ALU.add)
                if it == 0:
                    nc.vector.tensor_copy(bl[:, p, :], l3)
                else:
                    nc.vector.tensor_tensor(bl[:, p, :], bl[:, p, :], l3,
                                            op=ALU.add)

        def softmax():
            """e := c = softmax_o(bl); c -> c_t (i-partitioned) via DMA-T."""
            nc.scalar.activation(e[:, :, :], bl[:, :, :], ACTF.Exp)
            nc.vector.tensor_tensor(zh, e[:, 0, :], e[:, 1, :], op=ALU.add)
            nc.vector.tensor_tensor(rz, e[:, 2, :], e[:, 3, :], op=ALU.add)
            nc.vector.tensor_tensor(zh, zh, e[:, 4, :], op=ALU.add)
            nc.vector.tensor_tensor(zh, zh, rz, op=ALU.add)
            for n in range(3):
                sl = slice(384 * n, 384 * (n + 1))
                zp = psC.tile([128, 384], F32, name="zswap", tag="miscps")
                nc.tensor.matmul(zp[0:64, :], cid2[64:128, :], zh[64:128, sl],
                                 start=True, stop=True, tile_position=(64, 0))
                nc.tensor.matmul(zp[64:128, :], cid2[0:64, :], zh[0:64, sl],
                                 start=True, stop=True, tile_position=(0, 64))
                nc.vector.tensor_tensor(zh[:, sl], zh[:, sl], zp, op=ALU.add)
            nc.vector.reciprocal_approx_fast(rz, zh)
            for o5 in range(5):
                nc.vector.tensor_tensor(e[:, o5, :], e[:, o5, :], rz,
                                        op=ALU.mult)
            for o in range(O):
                o5, hh = o % 5, o // 5
                for t in range(T):
                    nc.sync.dma_start_transpose(
                        out=c_t[:, t, o, :],
                        in_=e[64 * hh : 64 * hh + 64, o5,
                              128 * t : 128 * (t + 1)],
                    )

        # ========================= flow =========================
        import os as _os
        STAGE = int(_os.environ.get("FLOW_STAGE", "99"))
        for it in range(3):
            if it > 0 and STAGE < 4:
                break
            if it == 0:
                for pair in PAIRS:
                    m1_pair(pair, lambda o: ui, 0.1)
            else:
                for pair in PAIRS:
                    cus = {}
                    for o in pair:
                        cu = cupool.tile([128, T, D, B], BF16, name="cu",
                                         tag="cu")
                        nc.vector.tensor_tensor(
                            cu[:, :, :, :],
                            c_t[:, :, o, :].unsqueeze(2).broadcast_to(
                                [128, T, D, B]),
                            ui[:, :, :, :],
                            op=ALU.mult,
                        )
                        cus[o] = cu
                    m1_pair(pair, lambda o: cus[o], 1.0)
            if STAGE >= 1:
                squash()
            if it < 2 and STAGE >= 2:
                m2_b2(it)
                if STAGE >= 3:
                    softmax()

        if STAGE < 1:
            nc.vector.memset(v_sb.rearrange("p s b -> p (s b)"), 0.5)
        for o in range(O):
            g, sl = o % 4, o // 4
            tp = psC.tile([64, J], F32, name="vtp", tag="miscps")
            nc.tensor.transpose(tp, v_sb[32 * g : 32 * g + 16, sl, :],
                                cid16s[32 * g : 32 * g + 16, :],
                                tile_position=(32 * g, 0))
            nc.scalar.copy(v_t[:, o, :], tp)
        nc.sync.dma_start(out=vout_d[:, :, :], in_=v_t)

    nc.finalize()
    return nc


def _host_prep(u, weights):
    """Per-core input maps. u [512,1152,8] f32, weights [1152,10,16,8] f32."""
    W = np.asarray(weights, dtype=np.float32)
    u = np.asarray(u, dtype=np.float32)
    ws = np.ascontiguousarray(
        W.reshape(T, 128, O, J, D).transpose(1, 0, 4, 2, 3)
    ).astype(bfnp)  # [128, T, D, O, J]
    wt = W.transpose(1, 2, 3, 0)  # [o, j, d, i]
    wb = np.zeros((128, 3 * KFLAT), dtype=bfnp)
    for o in range(O):
        g, sl = o % 4, o // 4
        wb[32 * g : 32 * g + 16, sl * KFLAT : (sl + 1) * KFLAT] = (
            wt[o].reshape(J, KFLAT).astype(bfnp)
        )
    cid2 = np.concatenate([np.eye(64, dtype=np.float32)] * 2, axis=0)
    cid16s = np.zeros((128, 16), dtype=np.float32)
    for g in range(4):
        cid16s[32 * g : 32 * g + 16, :] = np.eye(16, dtype=np.float32)
    cones128 = np.ones((128, 1), dtype=np.float32)
    cones1 = np.ones((1, 16), dtype=np.float32)

    base = {
        "ws": ws, "wb": wb, "cid2": cid2, "cid16s": cid16s,
        "cones128": cones128, "cones1": cones1,
    }
    in_maps = []
    for c in range(N_CORES):
        uc = u[c * B : (c + 1) * B]  # [64, 1152, 8]
        ui = np.ascontiguousarray(
            uc.reshape(B, T, 128, D).transpose(2, 1, 3, 0)
        ).astype(bfnp)  # [128, T, D, B]
        urh = np.ascontiguousarray(uc.transpose(0, 2, 1)).reshape(B, KFLAT)
        ur = np.concatenate([urh, urh], axis=0).astype(bfnp)  # [128, KFLAT]
        in_maps.append({**base, "ui": ui, "ur": ur})
    return in_maps


def kernel(u, weights):
    if "nc" not in _cache:
        _cache["nc"] = build_nc()
    nc = _cache["nc"]
    in_maps = _host_prep(u, weights)
    res = run_bass_kernel_spmd(nc, in_maps, core_ids=list(range(N_CORES)))
    out = np.concatenate([res.results[c]["vout"] for c in range(N_CORES)], axis=0)
    return out.astype(np.float32)


if __name__ == "__main__":
    rng = np.random.default_rng(0)
    u = rng.standard_normal((512, 1152, 8), dtype=np.float32)
    w = (rng.standard_normal((1152, 10, 16, 8)) * 0.1).astype(np.float32)
    v = kernel(u, w)
    print("out", v.shape, v.dtype, np.abs(v).max())



# revision 9
# speedup vs baseline: 1.6367x; 1.6367x over previous
"""CapsNet dynamic-routing layer on 8 Trainium2 NeuronCores (Bass/Tile).

reference math (per batch element b):
  u_hat[b,i,o,j] = sum_d W[i,o,j,d] * u[b,i,d]        (never materialized)
  bl = 0; for r in 0..2:
    c = softmax_o(bl); s[b,o,j] = sum_i c*u_hat; v = squash(s)
    if r < 2: bl += sum_j u_hat*v
  return v  [B, 10, 16]

Distribution: pure data parallel, batch 512 -> 64 per core x 8 cores;
weights replicated.  Per-core: b=64, i=1152=9*128, o=10, j=16, d=8.

Layouts:
  s-matmuls (m1): stationary cu chunk [(i,d)=128, b=64], streamed
    W_s[(i,d)chunk, (o,j)] -> PSUM s[b=64, (o,j)=160].  Streaming the
    16-wide W slice per o instead of the 64-wide batch cuts PE column
    cycles ~4x vs the W-stationary orientation.
  agreement (m2): per o-pair q=(2q,2q+1), ONE matmul per 512-chunk with a
    block-diagonal v2 [32, 128] lhsT (j=16 rows per o, b-halves in
    columns) -> G for both o's in one 512-col stream.  Then
    ug = G (.) ur on DVE/Act/Pool (split), d-fold tree on DVE.
  softmax runs in b-partition space (bl [128=(b,h), 5, 1152]); exp'd
    slices are DMA-transposed to i-partition space where the o-sum,
    reciprocal and u*(1/Z) fold happen once, so no per-o normalize pass.
"""
import sys

sys.path.insert(0, "/opt/trn_rl_repo")

import numpy as np
import ml_dtypes
from contextlib import ExitStack

from concourse import bacc, mybir, hw_specs
from concourse.tile import TileContext
from concourse.bass_utils import run_bass_kernel_spmd

BF16 = mybir.dt.bfloat16
F32 = mybir.dt.float32
AX = mybir.AxisListType
ALU = mybir.AluOpType
ACTF = mybir.ActivationFunctionType
bfnp = ml_dtypes.bfloat16

B = 64
I = 1152
T = 9
O = 10
J = 16
D = 8
EPS = 1e-06
N_CORES = 8
KFLAT = D * I          # 9216 (d-major flat for m2)
NCH = KFLAT // 512     # 18
NCHU = T * D           # 72 chunks of 128 on the (i,d) contraction

# engine assignment knobs
DVE_NNS = frozenset({0, 7, 13})   # m2 chunks routed Act-drain + DVE-mult
POOL_OS = frozenset()             # cu multiplies done on GPSIMD

_cache = {}

# Route every activation through the one table set that has exp+ln+copy,
# so the ACT engine never reloads tables mid-kernel.
_KEEP_SET = "natural_log_exp_and_others"


def _patched_tables(arch):
    full = {k: set(v) for k, v in hw_specs.get_activation_tables(arch).items()}
    keep = full[_KEEP_SET]
    return {k: (v if k == _KEEP_SET else v - keep) for k, v in full.items()}


import os
if os.environ.get('ACT_PATCH', '1') == '1':
    bacc.get_activation_tables = _patched_tables


def _oslot(o):
    """o -> (pair q / bl slot, psum half h).  o = 2q+h for o<8; pair 4 = (8,9)."""
    if o < 8:
        return o // 2, o % 2
    return 4, o - 8


def build_nc():
    nc = bacc.Bacc()
    ws_d = nc.dram_tensor("ws", [128, NCHU, O * J], BF16, kind="ExternalInput")
    wba_d = nc.dram_tensor("wba", [128, KFLAT], BF16, kind="ExternalInput")
    wbb_d = nc.dram_tensor("wbb", [32, KFLAT], BF16, kind="ExternalInput")
    ui_d = nc.dram_tensor("ui", [128, T, D, B], BF16, kind="ExternalInput")
    ur_d = nc.dram_tensor("ur", [128, KFLAT], BF16, kind="ExternalInput")
    vout_d = nc.dram_tensor("vout", [B, O, J], F32, kind="ExternalOutput")

    with TileContext(nc) as tc, ExitStack() as ctx:
        static = ctx.enter_context(tc.tile_pool(name="static", bufs=1))
        work = ctx.enter_context(tc.tile_pool(name="work", bufs=1))
        cupool = ctx.enter_context(tc.tile_pool(name="cup", bufs=2))
        ugpool = ctx.enter_context(tc.tile_pool(name="ugp", bufs=2))
        psS = ctx.enter_context(tc.tile_pool(name="psS", bufs=2, space="PSUM"))
        psM = ctx.enter_context(tc.tile_pool(name="psM", bufs=4, space="PSUM"))

        ws = static.tile([128, NCHU, O * J], BF16, name="ws")
        wba = static.tile([128, KFLAT], BF16, name="wba")
        wbb = static.tile([32, KFLAT], BF16, name="wbb")
        ui = static.tile([128, T, D, B], BF16, name="ui")
        ur = static.tile([128, KFLAT], BF16, name="ur")
        # split input loads across the two hwdge queues: SP carries what
        # iteration 0 needs (ws, ui); Act carries the m2-side tensors.
        nc.sync.dma_start(out=ws, in_=ws_d[:, :, :])
        nc.sync.dma_start(out=ui, in_=ui_d[:, :, :, :])
        nc.scalar.dma_start(out=ur, in_=ur_d[:, :])
        nc.scalar.dma_start(out=wba, in_=wba_d[:, :])
        nc.scalar.dma_start(out=wbb, in_=wbb_d[:, :])

        bl = work.tile([128, 5, I], F32, name="bl")
        e = work.tile([128, 5, I], BF16, name="e")
        et = work.tile([128, T, O, B], BF16, name="et")
        zt = work.tile([128, T, B], BF16, name="zt")
        ztf = work.tile([128, T, B], F32, name="ztf")
        rz = work.tile([128, T, B], F32, name="rz")
        rzb = work.tile([128, T, B], BF16, name="rzb")
        uz = work.tile([128, T, D, B], BF16, name="uz")
        s_sb = work.tile([B, O, J], F32, name="s_sb")
        s2 = work.tile([B, O, J], F32, name="s2")
        sq = work.tile([B, O], F32, name="sq")
        t1 = work.tile([B, O], F32, name="t1")
        t2 = work.tile([B, O], F32, name="t2")
        den = work.tile([B, O], F32, name="den")
        rec = work.tile([B, O], F32, name="rec")
        wsc = work.tile([B, O], F32, name="wsc")
        v_sb = work.tile([B, O, J], F32, name="v_sb")
        v_bf = work.tile([B, O, J], BF16, name="v_bf")
        # vz panels: b-partition staging for the block-diag v2 transposes.
        # vz0 cols 32q..32q+16 = v[b, 2q, :]; vz1 cols 32q+16..32q+32 =
        # v[b, 2q+1, :]; vzb0 cols 0:16 = v[b, 8, :]; vzb1 cols 16:32 =
        # v[b, 9, :]; all other columns stay zero forever.
        vz0 = work.tile([B, 128], BF16, name="vz0")
        vz1 = work.tile([B, 128], BF16, name="vz1")
        vzb0 = work.tile([B, 128], BF16, name="vzb0")
        vzb1 = work.tile([B, 128], BF16, name="vzb1")
        v2a = work.tile([128, 128], BF16, name="v2a")
        v2b = work.tile([128, 128], BF16, name="v2b")
        eps1 = work.tile([B, 1], F32, name="eps1")
        nc.vector.memset(eps1, EPS)
        for z in (vz0, vz1, vzb0, vzb1):
            nc.vector.memset(z, 0.0)

        # ------------- m1: s accumulation -------------
        def m1_it0():
            """s = 0.1 * sum_(i,d) W u  for all 10 o at once."""
            ps = psS.tile([B, O * J], F32, name="sps", tag="sps")
            for ch in range(NCHU):
                t, d = divmod(ch, D)
                nc.tensor.matmul(
                    ps, ui[:, t, d, :], ws[:, ch, :],
                    start=(ch == 0), stop=(ch == NCHU - 1),
                )
            return ps

        def m1_iter():
            """s_o = sum_(i,d) W_o (c_o*u), per-o cu stationary."""
            ps = psS.tile([B, O * J], F32, name="sps", tag="sps")
            for o in range(O):
                cu = cupool.tile([128, T, D, B], BF16, name="cu", tag="cu")
                ebc = et[:, :, o, :].unsqueeze(2).broadcast_to([128, T, D, B])
                if o in POOL_OS:
                    nc.gpsimd.scalar_tensor_tensor(
                        cu, ebc, 1.0, uz, op0=ALU.mult, op1=ALU.mult)
                else:
                    nc.vector.tensor_tensor(cu, ebc, uz, op=ALU.mult)
                for ch in range(NCHU):
                    t, d = divmod(ch, D)
                    nc.tensor.matmul(
                        ps[:, J * o : J * (o + 1)],
                        cu[:, t, d, :],
                        ws[:, ch, J * o : J * (o + 1)],
                        start=(ch == 0), stop=(ch == NCHU - 1),
                    )
            return ps

        # ------------- squash + v2 build -------------
        def squash(ps, scale, it):
            sf = s_sb.rearrange("b o j -> b (o j)")
            s2f = s2.rearrange("b o j -> b (o j)")
            nc.scalar.mul(sf, ps, scale)
            nc.vector.tensor_tensor(s2f, sf, sf, op=ALU.mult)
            nc.vector.tensor_reduce(sq, s2, axis=AX.X, op=ALU.add)
            nc.scalar.activation(t1, sq, ACTF.Ln, bias=eps1)
            nc.scalar.activation(t2, t1, ACTF.Exp, scale=0.5)  # sqrt(sq+eps)
            nc.vector.tensor_scalar_add(den, sq, 1.0)
            nc.vector.tensor_tensor(den, den, t2, op=ALU.mult)
            nc.vector.reciprocal_approx_accurate(rec, den, t1)
            nc.vector.tensor_tensor(wsc, sq, rec, op=ALU.mult)
            nc.vector.tensor_tensor(
                v_sb, s_sb, wsc.unsqueeze(2).broadcast_to([B, O, J]),
                op=ALU.mult)
            if it == 2:
                nc.sync.dma_start(out=vout_d[:, :, :], in_=v_sb)
                return
            vf = v_bf.rearrange("b o j -> b (o j)")
            nc.vector.tensor_copy(vf, v_sb.rearrange("b o j -> b (o j)"))
            for q in range(4):
                nc.scalar.copy(vz0[:, 32 * q : 32 * q + 16], v_bf[:, 2 * q, :])
                nc.scalar.copy(vz1[:, 32 * q + 16 : 32 * q + 32],
                               v_bf[:, 2 * q + 1, :])
            nc.scalar.copy(vzb0[:, 0:16], v_bf[:, 8, :])
            nc.scalar.copy(vzb1[:, 16:32], v_bf[:, 9, :])
            nc.sync.dma_start_transpose(out=v2a[:, 0:64], in_=vz0)
            nc.sync.dma_start_transpose(out=v2a[:, 64:128], in_=vz1)
            nc.sync.dma_start_transpose(out=v2b[:, 0:64], in_=vzb0)
            nc.sync.dma_start_transpose(out=v2b[:, 64:128], in_=vzb1)

        # ------------- m2: agreement -> bl -------------
        def m2(it):
            for q in range(5):
                ug = ugpool.tile([128, KFLAT], BF16, name="ug", tag="ug")
                for nn in range(NCH):
                    ps = psM.tile([128, 512], F32, name="m2ps", tag="m2ps")
                    csl = slice(512 * nn, 512 * (nn + 1))
                    if q < 4:
                        nc.tensor.matmul(
                            ps, v2a[32 * q : 32 * q + 32, :],
                            wba[32 * q : 32 * q + 32, csl],
                            start=True, stop=True,
                            tile_position=(32 * q, 0),
                        )
                    else:
                        nc.tensor.matmul(
                            ps, v2b[0:32, :], wbb[:, csl],
                            start=True, stop=True,
                            tile_position=(0, 0),
                        )
                    if nn in DVE_NNS:
                        nc.scalar.copy(ug[:, csl], ps)
                        nc.vector.tensor_tensor(ug[:, csl], ug[:, csl],
                                                ur[:, csl], op=ALU.mult)
                    else:
                        nc.gpsimd.scalar_tensor_tensor(
                            ug[:, csl], ps, 1.0, ur[:, csl],
                            op0=ALU.mult, op1=ALU.mult)
                # d-fold tree (d-major flat: level k folds d, d+4 / d+2 / d+1)
                # level 1 on DVE (bf16 2x rate), the f32-tainted tail on Pool.
                nc.vector.tensor_tensor(ug[:, 0:2048], ug[:, 0:2048],
                                        ug[:, 4608:6656], op=ALU.add)
                nc.vector.tensor_tensor(ug[:, 2048:4608], ug[:, 2048:4608],
                                        ug[:, 6656:9216], op=ALU.add)
                nc.gpsimd.scalar_tensor_tensor(
                    ug[:, 0:2304], ug[:, 0:2304], 0.0, ug[:, 2304:4608],
                    op0=ALU.add, op1=ALU.add)
                if it == 0:
                    nc.gpsimd.scalar_tensor_tensor(
                        bl[:, q, :], ug[:, 0:I], 0.0, ug[:, I : 2 * I],
                        op0=ALU.add, op1=ALU.add)
                else:
                    tmp = ug[:, 2304 : 2304 + I]
                    nc.gpsimd.scalar_tensor_tensor(
                        tmp, ug[:, 0:I], 0.0, ug[:, I : 2 * I],
                        op0=ALU.add, op1=ALU.add)
                    nc.gpsimd.scalar_tensor_tensor(
                        bl[:, q, :], bl[:, q, :], 0.0, tmp,
                        op0=ALU.add, op1=ALU.add)

        # ------------- softmax (i-space) + u/Z fold -------------
        def softmax():
            for p in range(5):
                nc.scalar.activation(e[:, p, :], bl[:, p, :], ACTF.Exp)
            for o in range(O):
                sl, h = _oslot(o)
                for t in range(T):
                    nc.sync.dma_start_transpose(
                        out=et[:, t, o, :],
                        in_=e[64 * h : 64 * h + 64, sl,
                              128 * t : 128 * (t + 1)],
                    )
            nc.vector.tensor_tensor(zt, et[:, :, 0, :], et[:, :, 1, :],
                                    op=ALU.add)
            for o in range(2, O):
                nc.vector.tensor_tensor(zt, zt, et[:, :, o, :], op=ALU.add)
            nc.vector.tensor_copy(ztf, zt)
            nc.vector.reciprocal_approx_fast(rz, ztf)
            nc.vector.tensor_copy(rzb, rz)
            nc.vector.tensor_tensor(
                uz, ui, rzb.unsqueeze(2).broadcast_to([128, T, D, B]),
                op=ALU.mult)

        # ========================= flow =========================
        ps0 = m1_it0()
        squash(ps0, 0.1, 0)
        m2(0)
        softmax()
        ps1 = m1_iter()
        squash(ps1, 1.0, 1)
        m2(1)
        softmax()
        ps2 = m1_iter()
        squash(ps2, 1.0, 2)

    nc.finalize()
    return nc


def _host_prep(u, weights):
    """Per-core input maps. u [512,1152,8] f32, weights [1152,10,16,8] f32."""
    W = np.asarray(weights, dtype=np.float32)
    u = np.asarray(u, dtype=np.float32)
    # W_s[p, t*8+d, o*16+j] = W[t*128+p, o, j, d]
    ws = np.ascontiguousarray(
        W.reshape(T, 128, O, J, D).transpose(1, 0, 4, 2, 3)
    ).reshape(128, NCHU, O * J).astype(bfnp)
    # wb rows 32q+16h+j = W[o=2q+h][j, d-major flat]
    wt = W.transpose(1, 2, 3, 0).reshape(O, J, KFLAT)  # [o, j, (d,i)]
    wba = np.zeros((128, KFLAT), dtype=bfnp)
    for o in range(8):
        q, h = o // 2, o % 2
        wba[32 * q + 16 * h : 32 * q + 16 * h + 16, :] = wt[o].astype(bfnp)
    wbb = np.zeros((32, KFLAT), dtype=bfnp)
    for o in (8, 9):
        h = o - 8
        wbb[16 * h : 16 * h + 16, :] = wt[o].astype(bfnp)

    base = {"ws": ws, "wba": wba, "wbb": wbb}
    in_maps = []
    for c in range(N_CORES):
        uc = u[c * B : (c + 1) * B]  # [64, 1152, 8]
        ui = np.ascontiguousarray(
            uc.reshape(B, T, 128, D).transpose(2, 1, 3, 0)
        ).astype(bfnp)  # [128, T, D, B]
        urh = np.ascontiguousarray(uc.transpose(0, 2, 1)).reshape(B, KFLAT)
        ur = np.concatenate([urh, urh], axis=0).astype(bfnp)  # [128, KFLAT]
        in_maps.append({**base, "ui": ui, "ur": ur})
    return in_maps


def kernel(u, weights):
    if "nc" not in _cache:
        _cache["nc"] = build_nc()
    nc = _cache["nc"]
    in_maps = _host_prep(u, weights)
    res = run_bass_kernel_spmd(nc, in_maps, core_ids=list(range(N_CORES)))
    out = np.concatenate([res.results[c]["vout"] for c in range(N_CORES)], axis=0)
    return out.astype(np.float32)


if __name__ == "__main__":
    rng = np.random.default_rng(0)
    u = rng.standard_normal((512, 1152, 8), dtype=np.float32)
    w = (rng.standard_normal((1152, 10, 16, 8)) * 0.1).astype(np.float32)
    v = kernel(u, w)
    print("out", v.shape, v.dtype, np.abs(v).max())


# revision 59
# speedup vs baseline: 2.0698x; 1.2646x over previous
"""CapsNet dynamic-routing layer on 8 Trainium2 NeuronCores (Bass/Tile).

reference math (per batch element b):
  u_hat[b,i,o,j] = sum_d W[i,o,j,d] * u[b,i,d]        (never materialized)
  bl = 0; for r in 0..2:
    c = softmax_o(bl); s[b,o,j] = sum_i c*u_hat; v = squash(s)
    if r < 2: bl += sum_j u_hat*v
  return v  [B, 10, 16]

Distribution: pure data parallel, batch 512 -> 64 per core x 8 cores;
weights replicated.  Per-core: b=64, i=1152=9*128, o=10, j=16, d=8.

Layouts:
  s-matmuls (m1): stationary cu chunk [(i,d)=128, b=64], streamed
    W_s[(i,d)chunk, (o,j)] -> PSUM s[b=64, (o,j)=160].  Streaming the
    16-wide W slice per o instead of the 64-wide batch cuts PE column
    cycles ~4x vs the W-stationary orientation.
  agreement (m2): per o-pair q=(2q,2q+1), ONE matmul per 512-chunk with a
    block-diagonal v2 [32, 128] lhsT (j=16 rows per o, b-halves in
    columns) -> G for both o's in one 512-col stream.  Then
    ug = G (.) ur on DVE/Act/Pool (split), d-fold tree on DVE.
  softmax runs in b-partition space (bl [128=(b,h), 5, 1152]); exp'd
    slices are DMA-transposed to i-partition space where the o-sum,
    reciprocal and u*(1/Z) fold happen once, so no per-o normalize pass.
"""
import sys

sys.path.insert(0, "/opt/trn_rl_repo")

import numpy as np
import ml_dtypes
from contextlib import ExitStack

from concourse import bacc, mybir, hw_specs
from concourse.tile import TileContext
from concourse.bass_utils import run_bass_kernel_spmd

BF16 = mybir.dt.bfloat16
F32 = mybir.dt.float32
AX = mybir.AxisListType
ALU = mybir.AluOpType
ACTF = mybir.ActivationFunctionType
bfnp = ml_dtypes.bfloat16

B = 64
I = 1152
T = 9
O = 10
J = 16
D = 8
EPS = 1e-06
N_CORES = 8
KFLAT = D * I          # 9216 (d-major flat for m2)
NCH = KFLAT // 512     # 18
NCHU = T * D           # 72 chunks of 128 on the (i,d) contraction

# engine assignment knobs (GPSIMD cannot touch PSUM on real HW, so every
# Pool multiply reads SBUF after an Act drain; DVE can read PSUM directly)
DIRECT_NNS = (0, 9)                  # single chunks: DVE mult straight from PSUM
DIRECT_PAIR = (16, 17)               # chunk pair: DVE mult straight from PSUM
DRAIN_PAIRS = ((1, 2), (3, 4), (5, 6), (7, 8), (10, 11), (12, 13), (14, 15))
DVE_PAIRS = frozenset({(1, 2)})      # drained pairs multiplied on DVE (rest Pool)
POOL_OS = frozenset({1, 3, 5, 8})    # cu multiplies done on GPSIMD

_cache = {}

# Route every activation through the one table set that has exp+ln+copy,
# so the ACT engine never reloads tables mid-kernel.
_KEEP_SET = "natural_log_exp_and_others"


def _patched_tables(arch):
    full = {k: set(v) for k, v in hw_specs.get_activation_tables(arch).items()}
    keep = full[_KEEP_SET]
    return {k: (v if k == _KEEP_SET else v - keep) for k, v in full.items()}


import os
if os.environ.get('ACT_PATCH', '1') == '1':
    bacc.get_activation_tables = _patched_tables


def _oslot(o):
    """o -> (pair q / bl slot, psum half h).  o = 2q+h for o<8; pair 4 = (8,9)."""
    if o < 8:
        return o // 2, o % 2
    return 4, o - 8


def build_nc():
    nc = bacc.Bacc()
    ws_d = nc.dram_tensor("ws", [128, NCHU, O * J], BF16, kind="ExternalInput")
    wba_d = nc.dram_tensor("wba", [128, KFLAT], BF16, kind="ExternalInput")
    wbb_d = nc.dram_tensor("wbb", [32, KFLAT], BF16, kind="ExternalInput")
    ui_d = nc.dram_tensor("ui", [128, T, D, B], BF16, kind="ExternalInput")
    ur_d = nc.dram_tensor("ur", [128, KFLAT], BF16, kind="ExternalInput")
    id64_d = nc.dram_tensor("id64", [B, B], BF16, kind="ExternalInput")
    vout_d = nc.dram_tensor("vout", [B, O, J], F32, kind="ExternalOutput")

    with TileContext(nc) as tc, ExitStack() as ctx:
        static = ctx.enter_context(tc.tile_pool(name="static", bufs=1))
        work = ctx.enter_context(tc.tile_pool(name="work", bufs=1))
        cupool = ctx.enter_context(tc.tile_pool(name="cup", bufs=2))
        ugpool = ctx.enter_context(tc.tile_pool(name="ugp", bufs=2))
        psS = ctx.enter_context(tc.tile_pool(name="psS", bufs=1, space="PSUM"))
        psT = ctx.enter_context(tc.tile_pool(name="psT", bufs=1, space="PSUM"))
        psM = ctx.enter_context(tc.tile_pool(name="psM", bufs=2, space="PSUM"))
        psN = ctx.enter_context(tc.tile_pool(name="psN", bufs=2, space="PSUM"))

        ws = static.tile([128, NCHU, O * J], BF16, name="ws")
        wba = static.tile([128, KFLAT], BF16, name="wba")
        wbb = static.tile([32, KFLAT], BF16, name="wbb")
        ui = static.tile([128, T, D, B], BF16, name="ui")
        ur = static.tile([128, KFLAT], BF16, name="ur")
        # split input loads across the two hwdge queues: SP carries what
        # iteration 0 needs (ws, ui), interleaved in it0's chunk order so
        # the PE can start after the first slice; Act carries the m2-side
        # tensors.
        for k in range(4):
            csl = slice(18 * k, 18 * k + 9)
            csl2 = slice(18 * k + 9, 18 * (k + 1))
            tsl = slice((T * k) // 4, (T * (k + 1)) // 4)
            nc.sync.dma_start(out=ws[:, csl, :], in_=ws_d[:, csl, :])
            nc.scalar.dma_start(out=ws[:, csl2, :], in_=ws_d[:, csl2, :])
            nc.sync.dma_start(out=ui[:, tsl, :, :], in_=ui_d[:, tsl, :, :])
        nc.gpsimd.dma_start(out=wba[:, 0:4608], in_=wba_d[:, 0:4608])
        nc.gpsimd.dma_start(out=ur[:, 0:2048], in_=ur_d[:, 0:2048])
        nc.gpsimd.dma_start(out=wba[:, 4608:KFLAT], in_=wba_d[:, 4608:KFLAT])
        nc.gpsimd.dma_start(out=ur[:, 2048:KFLAT], in_=ur_d[:, 2048:KFLAT])
        nc.gpsimd.dma_start(out=wbb, in_=wbb_d[:, :])
        id64 = static.tile([B, B], BF16, name="id64")
        nc.sync.dma_start(out=id64, in_=id64_d[:, :])

        bl = work.tile([128, 5, I], F32, name="bl")
        e = work.tile([128, 5, I], BF16, name="e")
        et = work.tile([128, T, O, B], BF16, name="et")
        zt = work.tile([128, T, B], BF16, name="zt")
        ztp = work.tile([128, T, B], BF16, name="ztp")
        ztf = work.tile([128, T, B], F32, name="ztf")
        rz = work.tile([128, T, B], F32, name="rz")
        rzb = work.tile([128, T, B], BF16, name="rzb")
        uz = work.tile([128, T, D, B], BF16, name="uz")
        s_sb = work.tile([B, O, J], F32, name="s_sb")
        s2 = work.tile([B, O, J], F32, name="s2")
        sq = work.tile([B, O], F32, name="sq")
        t1 = work.tile([B, O], F32, name="t1")
        t2 = work.tile([B, O], F32, name="t2")
        den = work.tile([B, O], F32, name="den")
        rec = work.tile([B, O], F32, name="rec")
        wsc = work.tile([B, O], F32, name="wsc")
        v_sb = work.tile([B, O, J], F32, name="v_sb")
        # vz panels: b-partition staging for the block-diag v2 transposes.
        # vz0 cols 32q..32q+16 = v[b, 2q, :]; vz1 cols 32q+16..32q+32 =
        # v[b, 2q+1, :]; vzb0 cols 0:16 = v[b, 8, :]; vzb1 cols 16:32 =
        # v[b, 9, :]; all other columns stay zero forever.
        vz0 = work.tile([B, 128], BF16, name="vz0")
        vz1 = work.tile([B, 128], BF16, name="vz1")
        vzb0 = work.tile([B, 128], BF16, name="vzb0")
        vzb1 = work.tile([B, 128], BF16, name="vzb1")
        v2a = work.tile([128, 128], BF16, name="v2a")
        v2b = work.tile([128, 128], BF16, name="v2b")
        eps1 = work.tile([B, 1], F32, name="eps1")
        nc.vector.memset(eps1, EPS)
        for z in (vz0, vz1, vzb0, vzb1):
            nc.vector.memset(z, 0.0)

        # ------------- m1: s accumulation -------------
        def m1_it0():
            """s~ = sum_(i,d) W u for all 10 o at once (c=0.1 applied in
            squash).  Two alternating PSUM banks so consecutive matmuls
            pipeline past the PSUM access latency."""
            ps = psS.tile([B, O * J], F32, name="sps", tag="sps")
            for ch in range(NCHU):
                t, d = divmod(ch, D)
                nc.tensor.matmul(
                    ps, ui[:, t, d, :], ws[:, ch, :],
                    start=(ch == 0), stop=(ch == NCHU - 1),
                )
            return ps

        TBLK = ((0, 3), (3, 6), (6, 9))

        def m1_iter():
            """s_o = sum_(i,d) W_o (c_o*u), per-o cu stationary."""
            ps = psS.tile([B, O * J], F32, name="sps", tag="sps")
            for o in range(O):
                cu = cupool.tile([128, T, D, B], BF16, name="cu", tag="cu")
                if o in POOL_OS:
                    # GPSIMD stt is limited to 2D/3D APs: go per-t slice.
                    for t in range(T):
                        ebc = et[:, t, o, :].unsqueeze(1).broadcast_to(
                            [128, D, B])
                        nc.gpsimd.tensor_tensor(cu[:, t, :, :], ebc,
                                                uz[:, t, :, :], op=ALU.mult)
                else:
                    for t0, t1 in TBLK:
                        tb = slice(t0, t1)
                        ebc = et[:, tb, o, :].unsqueeze(2).broadcast_to(
                            [128, t1 - t0, D, B])
                        nc.vector.tensor_tensor(cu[:, tb, :, :], ebc,
                                                uz[:, tb, :, :], op=ALU.mult)
                for ch in range(NCHU):
                    t, d = divmod(ch, D)
                    nc.tensor.matmul(
                        ps[:, J * o : J * (o + 1)],
                        cu[:, t, d, :],
                        ws[:, ch, J * o : J * (o + 1)],
                        start=(ch == 0), stop=(ch == NCHU - 1),
                    )
            return ps

        # ------------- squash + v2 build -------------
        def squash(ps, scale, it):
            sf = s_sb.rearrange("b o j -> b (o j)")
            s2f = s2.rearrange("b o j -> b (o j)")
            nc.scalar.mul(sf, ps, scale)
            nc.vector.tensor_tensor(s2f, sf, sf, op=ALU.mult)
            nc.vector.tensor_reduce(sq, s2, axis=AX.X, op=ALU.add)
            nc.scalar.activation(t1, sq, ACTF.Ln, bias=eps1)
            nc.scalar.activation(t2, t1, ACTF.Exp, scale=0.5)  # sqrt(sq+eps)
            nc.vector.tensor_scalar_add(den, sq, 1.0)
            nc.vector.tensor_tensor(den, den, t2, op=ALU.mult)
            nc.vector.reciprocal_approx_accurate(rec, den, t1)
            nc.vector.tensor_tensor(wsc, sq, rec, op=ALU.mult)
            nc.vector.tensor_tensor(
                v_sb, s_sb, wsc.unsqueeze(2).broadcast_to([B, O, J]),
                op=ALU.mult)
            if it == 2:
                nc.sync.dma_start(out=vout_d[:, :, :], in_=v_sb)
                return
            for q in range(4):
                nc.vector.tensor_copy(vz0[:, 32 * q : 32 * q + 16],
                                      v_sb[:, 2 * q, :])
                nc.vector.tensor_copy(vz1[:, 32 * q + 16 : 32 * q + 32],
                                      v_sb[:, 2 * q + 1, :])
            nc.vector.tensor_copy(vzb0[:, 0:16], v_sb[:, 8, :])
            nc.vector.tensor_copy(vzb1[:, 16:32], v_sb[:, 9, :])
            # transpose the vz panels onto v2 via the PE (identity matmul)
            # instead of DMA transposes: no DMA queue contention and no
            # dge completion delay on the round-boundary critical path.
            pt = psT.tile([128, 256], F32, name="v2t", tag="v2t")
            nc.tensor.matmul(pt[:, 0:64], vz0, id64, start=True, stop=True)
            nc.tensor.matmul(pt[:, 64:128], vz1, id64, start=True, stop=True)
            nc.tensor.matmul(pt[0:32, 128:192], vzb0[:, 0:32], id64,
                             start=True, stop=True)
            nc.tensor.matmul(pt[0:32, 192:256], vzb1[:, 0:32], id64,
                             start=True, stop=True)
            nc.scalar.copy(v2a, pt[:, 0:128])
            nc.scalar.copy(v2b[0:32, :], pt[0:32, 128:256])

        # ------------- m2: agreement -> bl -------------
        def m2(it):
            for q in range(5):
                ug = ugpool.tile([128, KFLAT], BF16, name="ug", tag="ug")

                def mm(dst, nn):
                    csl = slice(512 * nn, 512 * (nn + 1))
                    if q < 4:
                        nc.tensor.matmul(
                            dst, v2a[32 * q : 32 * q + 32, :],
                            wba[32 * q : 32 * q + 32, csl],
                            start=True, stop=True,
                            tile_position=(32 * q, 0),
                        )
                    else:
                        nc.tensor.matmul(
                            dst, v2b[0:32, :], wbb[:, csl],
                            start=True, stop=True,
                            tile_position=(0, 0),
                        )

                for nn in DIRECT_NNS:
                    ps = psN.tile([128, 512], F32, name="m2d", tag="m2d")
                    mm(ps, nn)
                    csl = slice(512 * nn, 512 * (nn + 1))
                    nc.vector.tensor_tensor(ug[:, csl], ps, ur[:, csl],
                                            op=ALU.mult)
                pn = psN.tile([128, 512], F32, name="m2d", tag="m2d")
                mm(pn, DIRECT_PAIR[0])
                csl = slice(512 * DIRECT_PAIR[0], 512 * (DIRECT_PAIR[0] + 1))
                nc.vector.tensor_tensor(ug[:, csl], pn, ur[:, csl],
                                        op=ALU.mult)
                pn = psN.tile([128, 512], F32, name="m2d", tag="m2d")
                mm(pn, DIRECT_PAIR[1])
                csl = slice(512 * DIRECT_PAIR[1], 512 * (DIRECT_PAIR[1] + 1))
                nc.vector.tensor_tensor(ug[:, csl], pn, ur[:, csl],
                                        op=ALU.mult)
                for n1, n2 in DRAIN_PAIRS:
                    pp = psM.tile([128, 1024], F32, name="m2p", tag="m2p")
                    mm(pp[:, 0:512], n1)
                    mm(pp[:, 512:1024], n2)
                    csl = slice(512 * n1, 512 * (n2 + 1))
                    nc.scalar.copy(ug[:, csl], pp)
                    if (n1, n2) in DVE_PAIRS:
                        nc.vector.tensor_tensor(ug[:, csl], ug[:, csl],
                                                ur[:, csl], op=ALU.mult)
                    else:
                        nc.gpsimd.tensor_tensor(ug[:, csl], ug[:, csl],
                                                ur[:, csl], op=ALU.mult)
                # d-fold tree (d-major flat: level k folds d, d+4 / d+2 / d+1)
                # level 1 on DVE (bf16 2x rate), the f32-tainted tail on Pool.
                # The last pair (q=4) gates the next softmax, so its levels
                # are split DVE || Pool to cut the boundary latency.
                nc.vector.tensor_tensor(ug[:, 0:2048], ug[:, 0:2048],
                                        ug[:, 4608:6656], op=ALU.add)
                nc.vector.tensor_tensor(ug[:, 2048:4608], ug[:, 2048:4608],
                                        ug[:, 6656:9216], op=ALU.add)
                nc.vector.tensor_tensor(ug[:, 0:2304], ug[:, 0:2304],
                                        ug[:, 2304:4608], op=ALU.add)
                if it == 0:
                    nc.gpsimd.tensor_tensor(bl[:, q, :], ug[:, 0:I],
                                            ug[:, I : 2 * I], op=ALU.add)
                else:
                    tmp = ug[:, 2304 : 2304 + I]
                    nc.gpsimd.tensor_tensor(tmp, ug[:, 0:I],
                                            ug[:, I : 2 * I], op=ALU.add)
                    nc.gpsimd.tensor_tensor(bl[:, q, :], bl[:, q, :], tmp,
                                            op=ALU.add)

        # ------------- softmax (i-space) + u/Z fold -------------
        def softmax():
            # slot 4 (the last pair) is exp'd in t-thirds so the t-blocked
            # tail below can start before the whole slot is done.
            for p in range(4):
                nc.scalar.activation(e[:, p, :], bl[:, p, :], ACTF.Exp)
            for t0, t1 in TBLK:
                nc.scalar.activation(e[:, 4, 128 * t0 : 128 * t1],
                                     bl[:, 4, 128 * t0 : 128 * t1], ACTF.Exp)
            for o in range(O):
                sl, h = _oslot(o)
                for t in range(T):
                    nc.sync.dma_start_transpose(
                        out=et[:, t, o, :],
                        in_=e[64 * h : 64 * h + 64, sl,
                              128 * t : 128 * (t + 1)],
                    )
            # Per t-block: DVE sums o={0,2,4,6,8,9} (late pairs last), Pool
            # sums o={1,3,5,7}, DVE merges, recip, 1/Z fold into u.
            for t0, t1 in TBLK:
                tb = slice(t0, t1)
                nc.vector.tensor_tensor(zt[:, tb, :], et[:, tb, 0, :],
                                        et[:, tb, 2, :], op=ALU.add)
                for o in (4, 6, 8, 9):
                    nc.vector.tensor_tensor(zt[:, tb, :], zt[:, tb, :],
                                            et[:, tb, o, :], op=ALU.add)
                nc.gpsimd.tensor_tensor(ztp[:, tb, :], et[:, tb, 1, :],
                                        et[:, tb, 3, :], op=ALU.add)
                for o in (5, 7):
                    nc.gpsimd.tensor_tensor(ztp[:, tb, :], ztp[:, tb, :],
                                            et[:, tb, o, :], op=ALU.add)
                nc.vector.tensor_tensor(ztf[:, tb, :], zt[:, tb, :],
                                        ztp[:, tb, :], op=ALU.add)
                nc.vector.reciprocal_approx_fast(rz[:, tb, :], ztf[:, tb, :])
                nc.vector.tensor_copy(rzb[:, tb, :], rz[:, tb, :])
                nc.vector.tensor_tensor(
                    uz[:, tb, :, :], ui[:, tb, :, :],
                    rzb[:, tb, :].unsqueeze(2).broadcast_to(
                        [128, t1 - t0, D, B]),
                    op=ALU.mult)

        # ========================= flow =========================
        ps0 = m1_it0()
        squash(ps0, 0.1, 0)
        m2(0)
        softmax()
        ps1 = m1_iter()
        squash(ps1, 1.0, 1)
        m2(1)
        softmax()
        ps2 = m1_iter()
        squash(ps2, 1.0, 2)

    nc.finalize()
    return nc


def _host_prep(u, weights):
    """Per-core input maps. u [512,1152,8] f32, weights [1152,10,16,8] f32."""
    W = np.asarray(weights, dtype=np.float32)
    u = np.asarray(u, dtype=np.float32)
    # W_s[p, t*8+d, o*16+j] = W[t*128+p, o, j, d]
    ws = np.ascontiguousarray(
        W.reshape(T, 128, O, J, D).transpose(1, 0, 4, 2, 3)
    ).reshape(128, NCHU, O * J).astype(bfnp)
    # wb rows 32q+16h+j = W[o=2q+h][j, d-major flat]
    wt = W.transpose(1, 2, 3, 0).reshape(O, J, KFLAT)  # [o, j, (d,i)]
    wba = np.zeros((128, KFLAT), dtype=bfnp)
    for o in range(8):
        q, h = o // 2, o % 2
        wba[32 * q + 16 * h : 32 * q + 16 * h + 16, :] = wt[o].astype(bfnp)
    wbb = np.zeros((32, KFLAT), dtype=bfnp)
    for o in (8, 9):
        h = o - 8
        wbb[16 * h : 16 * h + 16, :] = wt[o].astype(bfnp)

    base = {"ws": ws, "wba": wba, "wbb": wbb,
            "id64": np.eye(B, dtype=bfnp)}
    in_maps = []
    for c in range(N_CORES):
        uc = u[c * B : (c + 1) * B]  # [64, 1152, 8]
        ui = np.ascontiguousarray(
            uc.reshape(B, T, 128, D).transpose(2, 1, 3, 0)
        ).astype(bfnp)  # [128, T, D, B]
        urh = np.ascontiguousarray(uc.transpose(0, 2, 1)).reshape(B, KFLAT)
        ur = np.concatenate([urh, urh], axis=0).astype(bfnp)  # [128, KFLAT]
        in_maps.append({**base, "ui": ui, "ur": ur})
    return in_maps


def kernel(u, weights):
    if "nc" not in _cache:
        _cache["nc"] = build_nc()
    nc = _cache["nc"]
    in_maps = _host_prep(u, weights)
    res = run_bass_kernel_spmd(nc, in_maps, core_ids=list(range(N_CORES)))
    out = np.concatenate([res.results[c]["vout"] for c in range(N_CORES)], axis=0)
    return out.astype(np.float32)


if __name__ == "__main__":
    rng = np.random.default_rng(0)
    u = rng.standard_normal((512, 1152, 8), dtype=np.float32)
    w = (rng.standard_normal((1152, 10, 16, 8)) * 0.1).astype(np.float32)
    v = kernel(u, w)
    print("out", v.shape, v.dtype, np.abs(v).max())


# revision 73
# speedup vs baseline: 2.1197x; 1.0241x over previous
"""CapsNet dynamic-routing layer on 8 Trainium2 NeuronCores (Bass/Tile).

reference math (per batch element b):
  u_hat[b,i,o,j] = sum_d W[i,o,j,d] * u[b,i,d]        (never materialized)
  bl = 0; for r in 0..2:
    c = softmax_o(bl); s[b,o,j] = sum_i c*u_hat; v = squash(s)
    if r < 2: bl += sum_j u_hat*v
  return v  [B, 10, 16]

Distribution: pure data parallel, batch 512 -> 64 per core x 8 cores;
weights replicated.  Per-core: b=64, i=1152=9*128, o=10, j=16, d=8.

Layouts:
  s-matmuls (m1): stationary cu chunk [(i,d)=128, b=64], streamed
    W_s[(i,d)chunk, (o,j)] -> PSUM s[b=64, (o,j)=160].  Streaming the
    16-wide W slice per o instead of the 64-wide batch cuts PE column
    cycles ~4x vs the W-stationary orientation.
  agreement (m2): per o-pair q=(2q,2q+1), ONE matmul per 512-chunk with a
    block-diagonal v2 [32, 128] lhsT (j=16 rows per o, b-halves in
    columns) -> G for both o's in one 512-col stream.  Then
    ug = G (.) ur on DVE/Act/Pool (split), d-fold tree on DVE.
  softmax runs in b-partition space (bl [128=(b,h), 5, 1152]); exp'd
    slices are DMA-transposed to i-partition space where the o-sum,
    reciprocal and u*(1/Z) fold happen once, so no per-o normalize pass.
"""
import sys

sys.path.insert(0, "/opt/trn_rl_repo")

import numpy as np
import ml_dtypes
from contextlib import ExitStack

from concourse import bacc, mybir, hw_specs
from concourse.tile import TileContext
from concourse.bass_utils import run_bass_kernel_spmd

BF16 = mybir.dt.bfloat16
F32 = mybir.dt.float32
AX = mybir.AxisListType
ALU = mybir.AluOpType
ACTF = mybir.ActivationFunctionType
bfnp = ml_dtypes.bfloat16

B = 64
I = 1152
T = 9
O = 10
J = 16
D = 8
EPS = 1e-06
N_CORES = 8
KFLAT = D * I          # 9216 (d-major flat for m2)
NCH = KFLAT // 512     # 18
NCHU = T * D           # 72 chunks of 128 on the (i,d) contraction

# engine assignment knobs (GPSIMD cannot touch PSUM on real HW, so every
# Pool multiply reads SBUF after an Act drain; DVE can read PSUM directly)
DIRECT_NNS = (0, 9, 16, 17)                  # single chunks: DVE mult straight from PSUM
DIRECT_PAIR = (16, 17)               # chunk pair: DVE mult straight from PSUM
DRAIN_PAIRS = ((1, 2), (3, 4), (5, 6), (7, 8), (10, 11), (12, 13), (14, 15))
DVE_PAIRS = frozenset({(1, 2), (10, 11)})      # drained pairs multiplied on DVE (rest Pool)
POOL_OS = frozenset({1, 3, 5, 8})    # cu multiplies done on GPSIMD

_cache = {}

# Route every activation through the one table set that has exp+ln+copy,
# so the ACT engine never reloads tables mid-kernel.
_KEEP_SET = "natural_log_exp_and_others"


def _patched_tables(arch):
    full = {k: set(v) for k, v in hw_specs.get_activation_tables(arch).items()}
    keep = full[_KEEP_SET]
    return {k: (v if k == _KEEP_SET else v - keep) for k, v in full.items()}


import os
if os.environ.get('ACT_PATCH', '1') == '1':
    bacc.get_activation_tables = _patched_tables


def _oslot(o):
    """o -> (pair q / bl slot, psum half h).  o = 2q+h for o<8; pair 4 = (8,9)."""
    if o < 8:
        return o // 2, o % 2
    return 4, o - 8


def build_nc():
    nc = bacc.Bacc()
    ws_d = nc.dram_tensor("ws", [128, NCHU, O * J], BF16, kind="ExternalInput")
    wba_d = nc.dram_tensor("wba", [128, KFLAT], BF16, kind="ExternalInput")
    wbb_d = nc.dram_tensor("wbb", [32, KFLAT], BF16, kind="ExternalInput")
    ui_d = nc.dram_tensor("ui", [128, T, D, B], BF16, kind="ExternalInput")
    ur_d = nc.dram_tensor("ur", [128, KFLAT], BF16, kind="ExternalInput")
    id64_d = nc.dram_tensor("id64", [B, B], BF16, kind="ExternalInput")
    vout_d = nc.dram_tensor("vout", [B, O, J], F32, kind="ExternalOutput")

    with TileContext(nc) as tc, ExitStack() as ctx:
        static = ctx.enter_context(tc.tile_pool(name="static", bufs=1))
        work = ctx.enter_context(tc.tile_pool(name="work", bufs=1))
        cupool = ctx.enter_context(tc.tile_pool(name="cup", bufs=2))
        ugpool = ctx.enter_context(tc.tile_pool(name="ugp", bufs=2))
        psS = ctx.enter_context(tc.tile_pool(name="psS", bufs=1, space="PSUM"))
        psT = ctx.enter_context(tc.tile_pool(name="psT", bufs=1, space="PSUM"))
        psM = ctx.enter_context(tc.tile_pool(name="psM", bufs=2, space="PSUM"))
        psN = ctx.enter_context(tc.tile_pool(name="psN", bufs=2, space="PSUM"))

        ws = static.tile([128, NCHU, O * J], BF16, name="ws")
        wba = static.tile([128, KFLAT], BF16, name="wba")
        wbb = static.tile([32, KFLAT], BF16, name="wbb")
        ui = static.tile([128, T, D, B], BF16, name="ui")
        ur = static.tile([128, KFLAT], BF16, name="ur")
        # split input loads across the two hwdge queues: SP carries what
        # iteration 0 needs (ws, ui), interleaved in it0's chunk order so
        # the PE can start after the first slice; Act carries the m2-side
        # tensors.
        for k in range(4):
            csl = slice(18 * k, 18 * k + 9)
            csl2 = slice(18 * k + 9, 18 * (k + 1))
            tsl = slice((T * k) // 4, (T * (k + 1)) // 4)
            nc.sync.dma_start(out=ws[:, csl, :], in_=ws_d[:, csl, :])
            nc.scalar.dma_start(out=ws[:, csl2, :], in_=ws_d[:, csl2, :])
            nc.sync.dma_start(out=ui[:, tsl, :, :], in_=ui_d[:, tsl, :, :])
        nc.gpsimd.dma_start(out=wba[:, 0:4608], in_=wba_d[:, 0:4608])
        nc.gpsimd.dma_start(out=ur[:, 0:2048], in_=ur_d[:, 0:2048])
        nc.gpsimd.dma_start(out=wba[:, 4608:KFLAT], in_=wba_d[:, 4608:KFLAT])
        nc.gpsimd.dma_start(out=ur[:, 2048:KFLAT], in_=ur_d[:, 2048:KFLAT])
        nc.gpsimd.dma_start(out=wbb, in_=wbb_d[:, :])
        id64 = static.tile([B, B], BF16, name="id64")
        nc.sync.dma_start(out=id64, in_=id64_d[:, :])

        bl = work.tile([128, 5, I], F32, name="bl")
        e = work.tile([128, 5, I], BF16, name="e")
        et = work.tile([128, T, O, B], BF16, name="et")
        zt = work.tile([128, T, B], BF16, name="zt")
        ztp = work.tile([128, T, B], BF16, name="ztp")
        ztf = work.tile([128, T, B], F32, name="ztf")
        rz = work.tile([128, T, B], F32, name="rz")
        rzb = work.tile([128, T, B], BF16, name="rzb")
        uz = work.tile([128, T, D, B], BF16, name="uz")
        s_sb = work.tile([B, O, J], F32, name="s_sb")
        s2 = work.tile([B, O, J], F32, name="s2")
        sq = work.tile([B, O], F32, name="sq")
        t1 = work.tile([B, O], F32, name="t1")
        t2 = work.tile([B, O], F32, name="t2")
        den = work.tile([B, O], F32, name="den")
        rec = work.tile([B, O], F32, name="rec")
        wsc = work.tile([B, O], F32, name="wsc")
        v_sb = work.tile([B, O, J], F32, name="v_sb")
        # vz panels: b-partition staging for the block-diag v2 transposes.
        # vz0 cols 32q..32q+16 = v[b, 2q, :]; vz1 cols 32q+16..32q+32 =
        # v[b, 2q+1, :]; vzb0 cols 0:16 = v[b, 8, :]; vzb1 cols 16:32 =
        # v[b, 9, :]; all other columns stay zero forever.
        vz0 = work.tile([B, 128], BF16, name="vz0")
        vz1 = work.tile([B, 128], BF16, name="vz1")
        vzb0 = work.tile([B, 128], BF16, name="vzb0")
        vzb1 = work.tile([B, 128], BF16, name="vzb1")
        v2a = work.tile([128, 128], BF16, name="v2a")
        v2b = work.tile([128, 128], BF16, name="v2b")
        eps1 = work.tile([B, 1], F32, name="eps1")
        nc.vector.memset(eps1, EPS)
        for z in (vz0, vz1, vzb0, vzb1):
            nc.vector.memset(z, 0.0)

        # ------------- m1: s accumulation -------------
        def m1_it0():
            """s~ = sum_(i,d) W u for all 10 o at once (c=0.1 applied in
            squash).  Two alternating PSUM banks so consecutive matmuls
            pipeline past the PSUM access latency."""
            ps = psS.tile([B, O * J], F32, name="sps", tag="sps")
            for ch in range(NCHU):
                t, d = divmod(ch, D)
                nc.tensor.matmul(
                    ps, ui[:, t, d, :], ws[:, ch, :],
                    start=(ch == 0), stop=(ch == NCHU - 1),
                )
            return ps

        TBLK = ((0, 3), (3, 6), (6, 9))

        def m1_iter():
            """s_o = sum_(i,d) W_o (c_o*u), per-o cu stationary."""
            ps = psS.tile([B, O * J], F32, name="sps", tag="sps")
            for o in range(O):
                cu = cupool.tile([128, T, D, B], BF16, name="cu", tag="cu")
                if o in POOL_OS:
                    # GPSIMD stt is limited to 2D/3D APs: go per-t slice.
                    for t in range(T):
                        ebc = et[:, t, o, :].unsqueeze(1).broadcast_to(
                            [128, D, B])
                        nc.gpsimd.tensor_tensor(cu[:, t, :, :], ebc,
                                                uz[:, t, :, :], op=ALU.mult)
                else:
                    for t0, t1 in TBLK:
                        tb = slice(t0, t1)
                        ebc = et[:, tb, o, :].unsqueeze(2).broadcast_to(
                            [128, t1 - t0, D, B])
                        nc.vector.tensor_tensor(cu[:, tb, :, :], ebc,
                                                uz[:, tb, :, :], op=ALU.mult)
                for ch in range(NCHU):
                    t, d = divmod(ch, D)
                    nc.tensor.matmul(
                        ps[:, J * o : J * (o + 1)],
                        cu[:, t, d, :],
                        ws[:, ch, J * o : J * (o + 1)],
                        start=(ch == 0), stop=(ch == NCHU - 1),
                    )
            return ps

        # ------------- squash + v2 build -------------
        def squash(ps, scale, it):
            sf = s_sb.rearrange("b o j -> b (o j)")
            s2f = s2.rearrange("b o j -> b (o j)")
            nc.scalar.mul(sf, ps, scale)
            nc.vector.tensor_tensor(s2f, sf, sf, op=ALU.mult)
            nc.vector.tensor_reduce(sq, s2, axis=AX.X, op=ALU.add)
            nc.scalar.activation(t1, sq, ACTF.Ln, bias=eps1)
            nc.scalar.activation(t2, t1, ACTF.Exp, scale=0.5)  # sqrt(sq+eps)
            nc.vector.tensor_scalar_add(den, sq, 1.0)
            nc.vector.tensor_tensor(den, den, t2, op=ALU.mult)
            nc.vector.reciprocal_approx_accurate(rec, den, t1)
            nc.vector.tensor_tensor(wsc, sq, rec, op=ALU.mult)
            nc.vector.tensor_tensor(
                v_sb, s_sb, wsc.unsqueeze(2).broadcast_to([B, O, J]),
                op=ALU.mult)
            if it == 2:
                nc.sync.dma_start(out=vout_d[:, :, :], in_=v_sb)
                return
            for q in range(4):
                nc.vector.tensor_copy(vz0[:, 32 * q : 32 * q + 16],
                                      v_sb[:, 2 * q, :])
                nc.vector.tensor_copy(vz1[:, 32 * q + 16 : 32 * q + 32],
                                      v_sb[:, 2 * q + 1, :])
            nc.vector.tensor_copy(vzb0[:, 0:16], v_sb[:, 8, :])
            nc.vector.tensor_copy(vzb1[:, 16:32], v_sb[:, 9, :])
            # transpose the vz panels onto v2 via the PE (identity matmul)
            # instead of DMA transposes: no DMA queue contention and no
            # dge completion delay on the round-boundary critical path.
            pt = psT.tile([128, 256], F32, name="v2t", tag="v2t")
            nc.tensor.matmul(pt[:, 0:64], vz0, id64, start=True, stop=True)
            nc.tensor.matmul(pt[:, 64:128], vz1, id64, start=True, stop=True)
            nc.tensor.matmul(pt[0:32, 128:192], vzb0[:, 0:32], id64,
                             start=True, stop=True)
            nc.tensor.matmul(pt[0:32, 192:256], vzb1[:, 0:32], id64,
                             start=True, stop=True)
            nc.scalar.copy(v2a, pt[:, 0:128])
            nc.scalar.copy(v2b[0:32, :], pt[0:32, 128:256])

        # ------------- m2: agreement -> bl -------------
        def m2(it):
            for q in range(5):
                ug = ugpool.tile([128, KFLAT], BF16, name="ug", tag="ug")

                def mm(dst, nn):
                    csl = slice(512 * nn, 512 * (nn + 1))
                    if q < 4:
                        nc.tensor.matmul(
                            dst, v2a[32 * q : 32 * q + 32, :],
                            wba[32 * q : 32 * q + 32, csl],
                            start=True, stop=True,
                            tile_position=(32 * q, 0),
                        )
                    else:
                        nc.tensor.matmul(
                            dst, v2b[0:32, :], wbb[:, csl],
                            start=True, stop=True,
                            tile_position=(0, 0),
                        )

                def drain_pair(n1, n2):
                    pp = psM.tile([128, 1024], F32, name="m2p", tag="m2p")
                    mm(pp[:, 0:512], n1)
                    mm(pp[:, 512:1024], n2)
                    csl = slice(512 * n1, 512 * (n2 + 1))
                    nc.scalar.copy(ug[:, csl], pp)
                    if (n1, n2) in DVE_PAIRS:
                        nc.vector.tensor_tensor(ug[:, csl], ug[:, csl],
                                                ur[:, csl], op=ALU.mult)
                    else:
                        nc.gpsimd.tensor_tensor(ug[:, csl], ug[:, csl],
                                                ur[:, csl], op=ALU.mult)

                def direct(nn):
                    ps = psN.tile([128, 512], F32, name="m2d", tag="m2d")
                    mm(ps, nn)
                    csl = slice(512 * nn, 512 * (nn + 1))
                    nc.vector.tensor_tensor(ug[:, csl], ps, ur[:, csl],
                                            op=ALU.mult)

                for nn in DIRECT_NNS:
                    direct(nn)
                for n1, n2 in DRAIN_PAIRS:
                    drain_pair(n1, n2)
                # d-fold tree (d-major flat: level k folds d, d+4 / d+2 / d+1)
                # level 1 on DVE (bf16 2x rate), the f32-tainted tail on Pool.
                # The last pair (q=4) gates the next softmax, so its levels
                # are split DVE || Pool to cut the boundary latency.
                nc.vector.tensor_tensor(ug[:, 0:2048], ug[:, 0:2048],
                                        ug[:, 4608:6656], op=ALU.add)
                nc.vector.tensor_tensor(ug[:, 2048:4608], ug[:, 2048:4608],
                                        ug[:, 6656:9216], op=ALU.add)
                nc.vector.tensor_tensor(ug[:, 0:2304], ug[:, 0:2304],
                                        ug[:, 2304:4608], op=ALU.add)
                if it == 0:
                    nc.gpsimd.tensor_tensor(bl[:, q, :], ug[:, 0:I],
                                            ug[:, I : 2 * I], op=ALU.add)
                else:
                    tmp = ug[:, 2304 : 2304 + I]
                    nc.gpsimd.tensor_tensor(tmp, ug[:, 0:I],
                                            ug[:, I : 2 * I], op=ALU.add)
                    nc.gpsimd.tensor_tensor(bl[:, q, :], bl[:, q, :], tmp,
                                            op=ALU.add)

        # ------------- softmax (i-space) + u/Z fold -------------
        def softmax():
            # slot 4 (the last pair) is exp'd in t-thirds so the t-blocked
            # tail below can start before the whole slot is done.
            for p in range(4):
                nc.scalar.activation(e[:, p, :], bl[:, p, :], ACTF.Exp)
            for t0, t1 in TBLK:
                nc.scalar.activation(e[:, 4, 128 * t0 : 128 * t1],
                                     bl[:, 4, 128 * t0 : 128 * t1], ACTF.Exp)
            for o in range(O):
                sl, h = _oslot(o)
                for t in range(T):
                    nc.sync.dma_start_transpose(
                        out=et[:, t, o, :],
                        in_=e[64 * h : 64 * h + 64, sl,
                              128 * t : 128 * (t + 1)],
                    )
            # Per t-block: DVE sums o={0,2,4,6,8,9} (late pairs last), Pool
            # sums o={1,3,5,7}, DVE merges, recip, 1/Z fold into u.
            for t0, t1 in TBLK:
                tb = slice(t0, t1)
                nc.vector.tensor_tensor(zt[:, tb, :], et[:, tb, 0, :],
                                        et[:, tb, 2, :], op=ALU.add)
                for o in (4, 6, 8, 9):
                    nc.vector.tensor_tensor(zt[:, tb, :], zt[:, tb, :],
                                            et[:, tb, o, :], op=ALU.add)
                nc.gpsimd.tensor_tensor(ztp[:, tb, :], et[:, tb, 1, :],
                                        et[:, tb, 3, :], op=ALU.add)
                for o in (5, 7):
                    nc.gpsimd.tensor_tensor(ztp[:, tb, :], ztp[:, tb, :],
                                            et[:, tb, o, :], op=ALU.add)
                nc.vector.tensor_tensor(ztf[:, tb, :], zt[:, tb, :],
                                        ztp[:, tb, :], op=ALU.add)
                nc.vector.reciprocal_approx_fast(rz[:, tb, :], ztf[:, tb, :])
                nc.vector.tensor_copy(rzb[:, tb, :], rz[:, tb, :])
                if t0 == 3:
                    for t in range(t0, t1):
                        nc.gpsimd.tensor_tensor(
                            uz[:, t, :, :], ui[:, t, :, :],
                            rzb[:, t, :].unsqueeze(1).broadcast_to(
                                [128, D, B]),
                            op=ALU.mult)
                else:
                    nc.vector.tensor_tensor(
                        uz[:, tb, :, :], ui[:, tb, :, :],
                        rzb[:, tb, :].unsqueeze(2).broadcast_to(
                            [128, t1 - t0, D, B]),
                        op=ALU.mult)

        # ========================= flow =========================
        ps0 = m1_it0()
        squash(ps0, 0.1, 0)
        m2(0)
        softmax()
        ps1 = m1_iter()
        squash(ps1, 1.0, 1)
        m2(1)
        softmax()
        ps2 = m1_iter()
        squash(ps2, 1.0, 2)

    nc.finalize()
    return nc


def _host_prep(u, weights):
    """Per-core input maps. u [512,1152,8] f32, weights [1152,10,16,8] f32."""
    W = np.asarray(weights, dtype=np.float32)
    u = np.asarray(u, dtype=np.float32)
    # W_s[p, t*8+d, o*16+j] = W[t*128+p, o, j, d]
    ws = np.ascontiguousarray(
        W.reshape(T, 128, O, J, D).transpose(1, 0, 4, 2, 3)
    ).reshape(128, NCHU, O * J).astype(bfnp)
    # wb rows 32q+16h+j = W[o=2q+h][j, d-major flat]
    wt = W.transpose(1, 2, 3, 0).reshape(O, J, KFLAT)  # [o, j, (d,i)]
    wba = np.zeros((128, KFLAT), dtype=bfnp)
    for o in range(8):
        q, h = o // 2, o % 2
        wba[32 * q + 16 * h : 32 * q + 16 * h + 16, :] = wt[o].astype(bfnp)
    wbb = np.zeros((32, KFLAT), dtype=bfnp)
    for o in (8, 9):
        h = o - 8
        wbb[16 * h : 16 * h + 16, :] = wt[o].astype(bfnp)

    base = {"ws": ws, "wba": wba, "wbb": wbb,
            "id64": np.eye(B, dtype=bfnp)}
    in_maps = []
    for c in range(N_CORES):
        uc = u[c * B : (c + 1) * B]  # [64, 1152, 8]
        ui = np.ascontiguousarray(
            uc.reshape(B, T, 128, D).transpose(2, 1, 3, 0)
        ).astype(bfnp)  # [128, T, D, B]
        urh = np.ascontiguousarray(uc.transpose(0, 2, 1)).reshape(B, KFLAT)
        ur = np.concatenate([urh, urh], axis=0).astype(bfnp)  # [128, KFLAT]
        in_maps.append({**base, "ui": ui, "ur": ur})
    return in_maps


def kernel(u, weights):
    if "nc" not in _cache:
        _cache["nc"] = build_nc()
    nc = _cache["nc"]
    in_maps = _host_prep(u, weights)
    res = run_bass_kernel_spmd(nc, in_maps, core_ids=list(range(N_CORES)))
    out = np.concatenate([res.results[c]["vout"] for c in range(N_CORES)], axis=0)
    return out.astype(np.float32)


if __name__ == "__main__":
    rng = np.random.default_rng(0)
    u = rng.standard_normal((512, 1152, 8), dtype=np.float32)
    w = (rng.standard_normal((1152, 10, 16, 8)) * 0.1).astype(np.float32)
    v = kernel(u, w)
    print("out", v.shape, v.dtype, np.abs(v).max())


# revision 76
# speedup vs baseline: 2.1367x; 1.0080x over previous
"""CapsNet dynamic-routing layer on 8 Trainium2 NeuronCores (Bass/Tile).

reference math (per batch element b):
  u_hat[b,i,o,j] = sum_d W[i,o,j,d] * u[b,i,d]        (never materialized)
  bl = 0; for r in 0..2:
    c = softmax_o(bl); s[b,o,j] = sum_i c*u_hat; v = squash(s)
    if r < 2: bl += sum_j u_hat*v
  return v  [B, 10, 16]

Distribution: pure data parallel, batch 512 -> 64 per core x 8 cores;
weights replicated.  Per-core: b=64, i=1152=9*128, o=10, j=16, d=8.

Layouts:
  s-matmuls (m1): stationary cu chunk [(i,d)=128, b=64], streamed
    W_s[(i,d)chunk, (o,j)] -> PSUM s[b=64, (o,j)=160].  Streaming the
    16-wide W slice per o instead of the 64-wide batch cuts PE column
    cycles ~4x vs the W-stationary orientation.
  agreement (m2): per o-pair q=(2q,2q+1), ONE matmul per 512-chunk with a
    block-diagonal v2 [32, 128] lhsT (j=16 rows per o, b-halves in
    columns) -> G for both o's in one 512-col stream.  Then
    ug = G (.) ur on DVE/Act/Pool (split), d-fold tree on DVE.
  softmax runs in b-partition space (bl [128=(b,h), 5, 1152]); exp'd
    slices are DMA-transposed to i-partition space where the o-sum,
    reciprocal and u*(1/Z) fold happen once, so no per-o normalize pass.
"""
import sys

sys.path.insert(0, "/opt/trn_rl_repo")

import numpy as np
import ml_dtypes
from contextlib import ExitStack

from concourse import bacc, mybir, hw_specs
from concourse.tile import TileContext
from concourse.bass_utils import run_bass_kernel_spmd

BF16 = mybir.dt.bfloat16
F32 = mybir.dt.float32
AX = mybir.AxisListType
ALU = mybir.AluOpType
ACTF = mybir.ActivationFunctionType
bfnp = ml_dtypes.bfloat16

B = 64
I = 1152
T = 9
O = 10
J = 16
D = 8
EPS = 1e-06
N_CORES = 8
KFLAT = D * I          # 9216 (d-major flat for m2)
NCH = KFLAT // 512     # 18
NCHU = T * D           # 72 chunks of 128 on the (i,d) contraction

# engine assignment knobs (GPSIMD cannot touch PSUM on real HW, so every
# Pool multiply reads SBUF after an Act drain; DVE can read PSUM directly)
DIRECT_NNS = (0, 9, 16, 17)                  # single chunks: DVE mult straight from PSUM
DIRECT_PAIR = (16, 17)               # chunk pair: DVE mult straight from PSUM
DRAIN_PAIRS = ((1, 2), (3, 4), (5, 6), (7, 8), (10, 11), (12, 13), (14, 15))
DVE_PAIRS = frozenset({(1, 2), (10, 11)})      # drained pairs multiplied on DVE (rest Pool)
POOL_OS = frozenset({1, 3, 5, 8})    # cu multiplies done on GPSIMD

_cache = {}

# Route every activation through the one table set that has exp+ln+copy,
# so the ACT engine never reloads tables mid-kernel.
_KEEP_SET = "natural_log_exp_and_others"


def _patched_tables(arch):
    full = {k: set(v) for k, v in hw_specs.get_activation_tables(arch).items()}
    keep = full[_KEEP_SET]
    return {k: (v if k == _KEEP_SET else v - keep) for k, v in full.items()}


import os
if os.environ.get('ACT_PATCH', '1') == '1':
    bacc.get_activation_tables = _patched_tables


def _oslot(o):
    """o -> (pair q / bl slot, psum half h).  o = 2q+h for o<8; pair 4 = (8,9)."""
    if o < 8:
        return o // 2, o % 2
    return 4, o - 8


def build_nc():
    nc = bacc.Bacc()
    ws_d = nc.dram_tensor("ws", [128, NCHU, O * J], BF16, kind="ExternalInput")
    wba_d = nc.dram_tensor("wba", [128, KFLAT], BF16, kind="ExternalInput")
    wbb_d = nc.dram_tensor("wbb", [32, KFLAT], BF16, kind="ExternalInput")
    ui_d = nc.dram_tensor("ui", [128, T, D, B], BF16, kind="ExternalInput")
    ur_d = nc.dram_tensor("ur", [128, KFLAT], BF16, kind="ExternalInput")
    id64_d = nc.dram_tensor("id64", [B, B], BF16, kind="ExternalInput")
    vout_d = nc.dram_tensor("vout", [B, O, J], F32, kind="ExternalOutput")

    with TileContext(nc) as tc, ExitStack() as ctx:
        static = ctx.enter_context(tc.tile_pool(name="static", bufs=1))
        work = ctx.enter_context(tc.tile_pool(name="work", bufs=1))
        cupool = ctx.enter_context(tc.tile_pool(name="cup", bufs=2))
        ugpool = ctx.enter_context(tc.tile_pool(name="ugp", bufs=2))
        psS = ctx.enter_context(tc.tile_pool(name="psS", bufs=1, space="PSUM"))
        psT = ctx.enter_context(tc.tile_pool(name="psT", bufs=1, space="PSUM"))
        psM = ctx.enter_context(tc.tile_pool(name="psM", bufs=2, space="PSUM"))
        psN = ctx.enter_context(tc.tile_pool(name="psN", bufs=2, space="PSUM"))

        ws = static.tile([128, NCHU, O * J], BF16, name="ws")
        wba = static.tile([128, KFLAT], BF16, name="wba")
        wbb = static.tile([32, KFLAT], BF16, name="wbb")
        ui = static.tile([128, T, D, B], BF16, name="ui")
        ur = static.tile([128, KFLAT], BF16, name="ur")
        # split input loads across the two hwdge queues: SP carries what
        # iteration 0 needs (ws, ui), interleaved in it0's chunk order so
        # the PE can start after the first slice; Act carries the m2-side
        # tensors.
        for k in range(4):
            csl = slice(18 * k, 18 * k + 9)
            csl2 = slice(18 * k + 9, 18 * (k + 1))
            tsl = slice((T * k) // 4, (T * (k + 1)) // 4)
            nc.sync.dma_start(out=ws[:, csl, :], in_=ws_d[:, csl, :])
            nc.scalar.dma_start(out=ws[:, csl2, :], in_=ws_d[:, csl2, :])
            nc.sync.dma_start(out=ui[:, tsl, :, :], in_=ui_d[:, tsl, :, :])
        nc.gpsimd.dma_start(out=wba[:, 0:4608], in_=wba_d[:, 0:4608])
        nc.gpsimd.dma_start(out=ur[:, 0:4608], in_=ur_d[:, 0:4608])
        nc.gpsimd.dma_start(out=wba[:, 4608:KFLAT], in_=wba_d[:, 4608:KFLAT])
        nc.gpsimd.dma_start(out=ur[:, 4608:KFLAT], in_=ur_d[:, 4608:KFLAT])
        nc.gpsimd.dma_start(out=wbb, in_=wbb_d[:, :])
        id64 = static.tile([B, B], BF16, name="id64")
        nc.sync.dma_start(out=id64, in_=id64_d[:, :])

        bl = work.tile([128, 5, I], F32, name="bl")
        e = work.tile([128, 5, I], BF16, name="e")
        et = work.tile([128, T, O, B], BF16, name="et")
        zt = work.tile([128, T, B], BF16, name="zt")
        ztp = work.tile([128, T, B], BF16, name="ztp")
        ztf = work.tile([128, T, B], F32, name="ztf")
        rz = work.tile([128, T, B], F32, name="rz")
        rzb = work.tile([128, T, B], BF16, name="rzb")
        uz = work.tile([128, T, D, B], BF16, name="uz")
        s_sb = work.tile([B, O, J], F32, name="s_sb")
        s2 = work.tile([B, O, J], F32, name="s2")
        sq = work.tile([B, O], F32, name="sq")
        t1 = work.tile([B, O], F32, name="t1")
        t2 = work.tile([B, O], F32, name="t2")
        den = work.tile([B, O], F32, name="den")
        rec = work.tile([B, O], F32, name="rec")
        wsc = work.tile([B, O], F32, name="wsc")
        v_sb = work.tile([B, O, J], F32, name="v_sb")
        # vz panels: b-partition staging for the block-diag v2 transposes.
        # vz0 cols 32q..32q+16 = v[b, 2q, :]; vz1 cols 32q+16..32q+32 =
        # v[b, 2q+1, :]; vzb0 cols 0:16 = v[b, 8, :]; vzb1 cols 16:32 =
        # v[b, 9, :]; all other columns stay zero forever.
        vz0 = work.tile([B, 128], BF16, name="vz0")
        vz1 = work.tile([B, 128], BF16, name="vz1")
        vzb0 = work.tile([B, 128], BF16, name="vzb0")
        vzb1 = work.tile([B, 128], BF16, name="vzb1")
        v2a = work.tile([128, 128], BF16, name="v2a")
        v2b = work.tile([128, 128], BF16, name="v2b")
        eps1 = work.tile([B, 1], F32, name="eps1")
        nc.vector.memset(eps1, EPS)
        for z in (vz0, vz1, vzb0, vzb1):
            nc.vector.memset(z, 0.0)

        # ------------- m1: s accumulation -------------
        def m1_it0():
            """s~ = sum_(i,d) W u for all 10 o at once (c=0.1 applied in
            squash).  Two alternating PSUM banks so consecutive matmuls
            pipeline past the PSUM access latency."""
            ps = psS.tile([B, O * J], F32, name="sps", tag="sps")
            for ch in range(NCHU):
                t, d = divmod(ch, D)
                nc.tensor.matmul(
                    ps, ui[:, t, d, :], ws[:, ch, :],
                    start=(ch == 0), stop=(ch == NCHU - 1),
                )
            return ps

        TBLK = ((0, 3), (3, 6), (6, 9))

        def m1_iter():
            """s_o = sum_(i,d) W_o (c_o*u), per-o cu stationary."""
            ps = psS.tile([B, O * J], F32, name="sps", tag="sps")
            for o in range(O):
                cu = cupool.tile([128, T, D, B], BF16, name="cu", tag="cu")
                if o in POOL_OS:
                    # GPSIMD stt is limited to 2D/3D APs: go per-t slice.
                    for t in range(T):
                        ebc = et[:, t, o, :].unsqueeze(1).broadcast_to(
                            [128, D, B])
                        nc.gpsimd.tensor_tensor(cu[:, t, :, :], ebc,
                                                uz[:, t, :, :], op=ALU.mult)
                else:
                    for t0, t1 in TBLK:
                        tb = slice(t0, t1)
                        ebc = et[:, tb, o, :].unsqueeze(2).broadcast_to(
                            [128, t1 - t0, D, B])
                        nc.vector.tensor_tensor(cu[:, tb, :, :], ebc,
                                                uz[:, tb, :, :], op=ALU.mult)
                for ch in range(NCHU):
                    t, d = divmod(ch, D)
                    nc.tensor.matmul(
                        ps[:, J * o : J * (o + 1)],
                        cu[:, t, d, :],
                        ws[:, ch, J * o : J * (o + 1)],
                        start=(ch == 0), stop=(ch == NCHU - 1),
                    )
            return ps

        # ------------- squash + v2 build -------------
        def squash(ps, scale, it):
            sf = s_sb.rearrange("b o j -> b (o j)")
            s2f = s2.rearrange("b o j -> b (o j)")
            nc.scalar.mul(sf, ps, scale)
            nc.vector.tensor_tensor(s2f, sf, sf, op=ALU.mult)
            nc.vector.tensor_reduce(sq, s2, axis=AX.X, op=ALU.add)
            nc.scalar.activation(t1, sq, ACTF.Ln, bias=eps1)
            nc.scalar.activation(t2, t1, ACTF.Exp, scale=0.5)  # sqrt(sq+eps)
            nc.vector.tensor_scalar_add(den, sq, 1.0)
            nc.vector.tensor_tensor(den, den, t2, op=ALU.mult)
            nc.vector.reciprocal_approx_accurate(rec, den, t1)
            nc.vector.tensor_tensor(wsc, sq, rec, op=ALU.mult)
            nc.vector.tensor_tensor(
                v_sb, s_sb, wsc.unsqueeze(2).broadcast_to([B, O, J]),
                op=ALU.mult)
            if it == 2:
                nc.sync.dma_start(out=vout_d[:, :, :], in_=v_sb)
                return
            for q in range(4):
                nc.vector.tensor_copy(vz0[:, 32 * q : 32 * q + 16],
                                      v_sb[:, 2 * q, :])
                nc.vector.tensor_copy(vz1[:, 32 * q + 16 : 32 * q + 32],
                                      v_sb[:, 2 * q + 1, :])
            nc.vector.tensor_copy(vzb0[:, 0:16], v_sb[:, 8, :])
            nc.vector.tensor_copy(vzb1[:, 16:32], v_sb[:, 9, :])
            # transpose the vz panels onto v2 via the PE (identity matmul)
            # instead of DMA transposes: no DMA queue contention and no
            # dge completion delay on the round-boundary critical path.
            pt = psT.tile([128, 256], F32, name="v2t", tag="v2t")
            nc.tensor.matmul(pt[:, 0:64], vz0, id64, start=True, stop=True)
            nc.tensor.matmul(pt[:, 64:128], vz1, id64, start=True, stop=True)
            nc.tensor.matmul(pt[0:32, 128:192], vzb0[:, 0:32], id64,
                             start=True, stop=True)
            nc.tensor.matmul(pt[0:32, 192:256], vzb1[:, 0:32], id64,
                             start=True, stop=True)
            nc.scalar.copy(v2a, pt[:, 0:128])
            nc.scalar.copy(v2b[0:32, :], pt[0:32, 128:256])

        # ------------- m2: agreement -> bl -------------
        def m2(it):
            for q in range(5):
                ug = ugpool.tile([128, KFLAT], BF16, name="ug", tag="ug")

                def mm(dst, nn):
                    csl = slice(512 * nn, 512 * (nn + 1))
                    if q < 4:
                        nc.tensor.matmul(
                            dst, v2a[32 * q : 32 * q + 32, :],
                            wba[32 * q : 32 * q + 32, csl],
                            start=True, stop=True,
                            tile_position=(32 * q, 0),
                        )
                    else:
                        nc.tensor.matmul(
                            dst, v2b[0:32, :], wbb[:, csl],
                            start=True, stop=True,
                            tile_position=(0, 0),
                        )

                def drain_pair(n1, n2):
                    pp = psM.tile([128, 1024], F32, name="m2p", tag="m2p")
                    mm(pp[:, 0:512], n1)
                    mm(pp[:, 512:1024], n2)
                    csl = slice(512 * n1, 512 * (n2 + 1))
                    nc.scalar.copy(ug[:, csl], pp)
                    if (n1, n2) in DVE_PAIRS:
                        nc.vector.tensor_tensor(ug[:, csl], ug[:, csl],
                                                ur[:, csl], op=ALU.mult)
                    else:
                        nc.gpsimd.tensor_tensor(ug[:, csl], ug[:, csl],
                                                ur[:, csl], op=ALU.mult)

                def direct(nn):
                    ps = psN.tile([128, 512], F32, name="m2d", tag="m2d")
                    mm(ps, nn)
                    csl = slice(512 * nn, 512 * (nn + 1))
                    nc.vector.tensor_tensor(ug[:, csl], ps, ur[:, csl],
                                            op=ALU.mult)

                for nn in DIRECT_NNS:
                    direct(nn)
                for n1, n2 in DRAIN_PAIRS:
                    drain_pair(n1, n2)
                # d-fold tree (d-major flat: level k folds d, d+4 / d+2 / d+1)
                # level 1 on DVE (bf16 2x rate), the f32-tainted tail on Pool.
                # The last pair (q=4) gates the next softmax, so its levels
                # are split DVE || Pool to cut the boundary latency.
                nc.vector.tensor_tensor(ug[:, 0:2048], ug[:, 0:2048],
                                        ug[:, 4608:6656], op=ALU.add)
                nc.vector.tensor_tensor(ug[:, 2048:4608], ug[:, 2048:4608],
                                        ug[:, 6656:9216], op=ALU.add)
                nc.vector.tensor_tensor(ug[:, 0:2304], ug[:, 0:2304],
                                        ug[:, 2304:4608], op=ALU.add)
                if it == 0:
                    nc.gpsimd.tensor_tensor(bl[:, q, :], ug[:, 0:I],
                                            ug[:, I : 2 * I], op=ALU.add)
                else:
                    tmp = ug[:, 2304 : 2304 + I]
                    nc.gpsimd.tensor_tensor(tmp, ug[:, 0:I],
                                            ug[:, I : 2 * I], op=ALU.add)
                    nc.gpsimd.tensor_tensor(bl[:, q, :], bl[:, q, :], tmp,
                                            op=ALU.add)

        # ------------- softmax (i-space) + u/Z fold -------------
        def softmax():
            # slot 4 (the last pair) is exp'd in t-thirds so the t-blocked
            # tail below can start before the whole slot is done.
            for p in range(4):
                nc.scalar.activation(e[:, p, :], bl[:, p, :], ACTF.Exp)
            for t0, t1 in TBLK:
                nc.scalar.activation(e[:, 4, 128 * t0 : 128 * t1],
                                     bl[:, 4, 128 * t0 : 128 * t1], ACTF.Exp)
            for o in range(O):
                sl, h = _oslot(o)
                for t in range(T):
                    nc.sync.dma_start_transpose(
                        out=et[:, t, o, :],
                        in_=e[64 * h : 64 * h + 64, sl,
                              128 * t : 128 * (t + 1)],
                    )
            # Per t-block: DVE sums o={0,2,4,6,8,9} (late pairs last), Pool
            # sums o={1,3,5,7}, DVE merges, recip, 1/Z fold into u.
            for t0, t1 in TBLK:
                tb = slice(t0, t1)
                nc.vector.tensor_tensor(zt[:, tb, :], et[:, tb, 0, :],
                                        et[:, tb, 2, :], op=ALU.add)
                for o in (4, 6, 8, 9):
                    nc.vector.tensor_tensor(zt[:, tb, :], zt[:, tb, :],
                                            et[:, tb, o, :], op=ALU.add)
                nc.gpsimd.tensor_tensor(ztp[:, tb, :], et[:, tb, 1, :],
                                        et[:, tb, 3, :], op=ALU.add)
                for o in (5, 7):
                    nc.gpsimd.tensor_tensor(ztp[:, tb, :], ztp[:, tb, :],
                                            et[:, tb, o, :], op=ALU.add)
                nc.vector.tensor_tensor(ztf[:, tb, :], zt[:, tb, :],
                                        ztp[:, tb, :], op=ALU.add)
                nc.vector.reciprocal_approx_fast(rz[:, tb, :], ztf[:, tb, :])
                nc.vector.tensor_copy(rzb[:, tb, :], rz[:, tb, :])
                if t0 == 3:
                    for t in range(t0, t1):
                        nc.gpsimd.tensor_tensor(
                            uz[:, t, :, :], ui[:, t, :, :],
                            rzb[:, t, :].unsqueeze(1).broadcast_to(
                                [128, D, B]),
                            op=ALU.mult)
                else:
                    nc.vector.tensor_tensor(
                        uz[:, tb, :, :], ui[:, tb, :, :],
                        rzb[:, tb, :].unsqueeze(2).broadcast_to(
                            [128, t1 - t0, D, B]),
                        op=ALU.mult)

        # ========================= flow =========================
        ps0 = m1_it0()
        squash(ps0, 0.1, 0)
        m2(0)
        softmax()
        ps1 = m1_iter()
        squash(ps1, 1.0, 1)
        m2(1)
        softmax()
        ps2 = m1_iter()
        squash(ps2, 1.0, 2)

    nc.finalize()
    return nc


def _host_prep(u, weights):
    """Per-core input maps. u [512,1152,8] f32, weights [1152,10,16,8] f32."""
    W = np.asarray(weights, dtype=np.float32)
    u = np.asarray(u, dtype=np.float32)
    # W_s[p, t*8+d, o*16+j] = W[t*128+p, o, j, d]
    ws = np.ascontiguousarray(
        W.reshape(T, 128, O, J, D).transpose(1, 0, 4, 2, 3)
    ).reshape(128, NCHU, O * J).astype(bfnp)
    # wb rows 32q+16h+j = W[o=2q+h][j, d-major flat]
    wt = W.transpose(1, 2, 3, 0).reshape(O, J, KFLAT)  # [o, j, (d,i)]
    wba = np.zeros((128, KFLAT), dtype=bfnp)
    for o in range(8):
        q, h = o // 2, o % 2
        wba[32 * q + 16 * h : 32 * q + 16 * h + 16, :] = wt[o].astype(bfnp)
    wbb = np.zeros((32, KFLAT), dtype=bfnp)
    for o in (8, 9):
        h = o - 8
        wbb[16 * h : 16 * h + 16, :] = wt[o].astype(bfnp)

    base = {"ws": ws, "wba": wba, "wbb": wbb,
            "id64": np.eye(B, dtype=bfnp)}
    in_maps = []
    for c in range(N_CORES):
        uc = u[c * B : (c + 1) * B]  # [64, 1152, 8]
        ui = np.ascontiguousarray(
            uc.reshape(B, T, 128, D).transpose(2, 1, 3, 0)
        ).astype(bfnp)  # [128, T, D, B]
        urh = np.ascontiguousarray(uc.transpose(0, 2, 1)).reshape(B, KFLAT)
        ur = np.concatenate([urh, urh], axis=0).astype(bfnp)  # [128, KFLAT]
        in_maps.append({**base, "ui": ui, "ur": ur})
    return in_maps


def kernel(u, weights):
    if "nc" not in _cache:
        _cache["nc"] = build_nc()
    nc = _cache["nc"]
    in_maps = _host_prep(u, weights)
    res = run_bass_kernel_spmd(nc, in_maps, core_ids=list(range(N_CORES)))
    out = np.concatenate([res.results[c]["vout"] for c in range(N_CORES)], axis=0)
    return out.astype(np.float32)


if __name__ == "__main__":
    rng = np.random.default_rng(0)
    u = rng.standard_normal((512, 1152, 8), dtype=np.float32)
    w = (rng.standard_normal((1152, 10, 16, 8)) * 0.1).astype(np.float32)
    v = kernel(u, w)
    print("out", v.shape, v.dtype, np.abs(v).max())


# revision 90
# speedup vs baseline: 2.1497x; 1.0061x over previous
"""CapsNet dynamic-routing layer on 8 Trainium2 NeuronCores (Bass/Tile).

reference math (per batch element b):
  u_hat[b,i,o,j] = sum_d W[i,o,j,d] * u[b,i,d]        (never materialized)
  bl = 0; for r in 0..2:
    c = softmax_o(bl); s[b,o,j] = sum_i c*u_hat; v = squash(s)
    if r < 2: bl += sum_j u_hat*v
  return v  [B, 10, 16]

Distribution: pure data parallel, batch 512 -> 64 per core x 8 cores;
weights replicated.  Per-core: b=64, i=1152=9*128, o=10, j=16, d=8.

Layouts:
  s-matmuls (m1): stationary cu chunk [(i,d)=128, b=64], streamed
    W_s[(i,d)chunk, (o,j)] -> PSUM s[b=64, (o,j)=160].  Streaming the
    16-wide W slice per o instead of the 64-wide batch cuts PE column
    cycles ~4x vs the W-stationary orientation.
  agreement (m2): per o-pair q=(2q,2q+1), ONE matmul per 512-chunk with a
    block-diagonal v2 [32, 128] lhsT (j=16 rows per o, b-halves in
    columns) -> G for both o's in one 512-col stream.  Then
    ug = G (.) ur on DVE/Act/Pool (split), d-fold tree on DVE.
  softmax runs in b-partition space (bl [128=(b,h), 5, 1152]); exp'd
    slices are DMA-transposed to i-partition space where the o-sum,
    reciprocal and u*(1/Z) fold happen once, so no per-o normalize pass.
"""
import sys

sys.path.insert(0, "/opt/trn_rl_repo")

import numpy as np
import ml_dtypes
from contextlib import ExitStack

from concourse import bacc, mybir, hw_specs
from concourse.tile import TileContext
from concourse.bass_utils import run_bass_kernel_spmd

BF16 = mybir.dt.bfloat16
F32 = mybir.dt.float32
AX = mybir.AxisListType
ALU = mybir.AluOpType
ACTF = mybir.ActivationFunctionType
bfnp = ml_dtypes.bfloat16

B = 64
I = 1152
T = 9
O = 10
J = 16
D = 8
EPS = 1e-06
N_CORES = 8
KFLAT = D * I          # 9216 (d-major flat for m2)
NCH = KFLAT // 512     # 18
NCHU = T * D           # 72 chunks of 128 on the (i,d) contraction

# engine assignment knobs (GPSIMD cannot touch PSUM on real HW, so every
# Pool multiply reads SBUF after an Act drain; DVE can read PSUM directly)
DIRECT_NNS = (0, 9, 16, 17)                  # single chunks: DVE mult straight from PSUM
DIRECT_PAIR = (16, 17)               # chunk pair: DVE mult straight from PSUM
DRAIN_PAIRS = ((1, 2), (3, 4), (5, 6), (7, 8), (10, 11), (12, 13), (14, 15))
DVE_PAIRS = frozenset({(1, 2), (10, 11)})      # drained pairs multiplied on DVE (rest Pool)
POOL_OS = frozenset({1, 3, 6, 8})    # cu multiplies done on GPSIMD

_cache = {}

# Route every activation through the one table set that has exp+ln+copy,
# so the ACT engine never reloads tables mid-kernel.
_KEEP_SET = "natural_log_exp_and_others"


def _patched_tables(arch):
    full = {k: set(v) for k, v in hw_specs.get_activation_tables(arch).items()}
    keep = full[_KEEP_SET]
    return {k: (v if k == _KEEP_SET else v - keep) for k, v in full.items()}


import os
if os.environ.get('ACT_PATCH', '1') == '1':
    bacc.get_activation_tables = _patched_tables


def _oslot(o):
    """o -> (pair q / bl slot, psum half h).  o = 2q+h for o<8; pair 4 = (8,9)."""
    if o < 8:
        return o // 2, o % 2
    return 4, o - 8


def build_nc():
    nc = bacc.Bacc()
    ws_d = nc.dram_tensor("ws", [128, NCHU, O * J], BF16, kind="ExternalInput")
    wba_d = nc.dram_tensor("wba", [128, KFLAT], BF16, kind="ExternalInput")
    wbb_d = nc.dram_tensor("wbb", [32, KFLAT], BF16, kind="ExternalInput")
    ui_d = nc.dram_tensor("ui", [128, T, D, B], BF16, kind="ExternalInput")
    ur_d = nc.dram_tensor("ur", [128, KFLAT], BF16, kind="ExternalInput")
    id64_d = nc.dram_tensor("id64", [B, B], BF16, kind="ExternalInput")
    vout_d = nc.dram_tensor("vout", [B, O, J], F32, kind="ExternalOutput")

    with TileContext(nc) as tc, ExitStack() as ctx:
        static = ctx.enter_context(tc.tile_pool(name="static", bufs=1))
        work = ctx.enter_context(tc.tile_pool(name="work", bufs=1))
        cupool = ctx.enter_context(tc.tile_pool(name="cup", bufs=2))
        ugpool = ctx.enter_context(tc.tile_pool(name="ugp", bufs=2))
        psS = ctx.enter_context(tc.tile_pool(name="psS", bufs=1, space="PSUM"))
        psT = ctx.enter_context(tc.tile_pool(name="psT", bufs=1, space="PSUM"))
        psM = ctx.enter_context(tc.tile_pool(name="psM", bufs=2, space="PSUM"))
        psN = ctx.enter_context(tc.tile_pool(name="psN", bufs=2, space="PSUM"))

        ws = static.tile([128, NCHU, O * J], BF16, name="ws")
        wba = static.tile([128, KFLAT], BF16, name="wba")
        wbb = static.tile([32, KFLAT], BF16, name="wbb")
        ui = static.tile([128, T, D, B], BF16, name="ui")
        ur = static.tile([128, KFLAT], BF16, name="ur")
        # split input loads across the two hwdge queues: SP carries what
        # iteration 0 needs (ws, ui), interleaved in it0's chunk order so
        # the PE can start after the first slice; Act carries the m2-side
        # tensors.
        for k in range(4):
            csl = slice(18 * k, 18 * k + 9)
            csl2 = slice(18 * k + 9, 18 * (k + 1))
            tsl = slice((T * k) // 4, (T * (k + 1)) // 4)
            nc.sync.dma_start(out=ws[:, csl, :], in_=ws_d[:, csl, :])
            nc.scalar.dma_start(out=ws[:, csl2, :], in_=ws_d[:, csl2, :])
            if k % 2 == 0:
                nc.sync.dma_start(out=ui[:, tsl, :, :], in_=ui_d[:, tsl, :, :])
            else:
                nc.scalar.dma_start(out=ui[:, tsl, :, :],
                                    in_=ui_d[:, tsl, :, :])
        nc.gpsimd.dma_start(out=wba[:, 0:4608], in_=wba_d[:, 0:4608])
        nc.gpsimd.dma_start(out=ur[:, 0:4608], in_=ur_d[:, 0:4608])
        nc.gpsimd.dma_start(out=wba[:, 4608:KFLAT], in_=wba_d[:, 4608:KFLAT])
        nc.gpsimd.dma_start(out=ur[:, 4608:KFLAT], in_=ur_d[:, 4608:KFLAT])
        nc.gpsimd.dma_start(out=wbb, in_=wbb_d[:, :])
        id64 = static.tile([B, B], BF16, name="id64")
        nc.sync.dma_start(out=id64, in_=id64_d[:, :])

        bl = work.tile([128, 5, I], F32, name="bl")
        e = work.tile([128, 5, I], BF16, name="e")
        et = work.tile([128, T, O, B], BF16, name="et")
        zt = work.tile([128, T, B], BF16, name="zt")
        ztp = work.tile([128, T, B], BF16, name="ztp")
        ztf = work.tile([128, T, B], F32, name="ztf")
        rz = work.tile([128, T, B], F32, name="rz")
        rzb = work.tile([128, T, B], BF16, name="rzb")
        uz = work.tile([128, T, D, B], BF16, name="uz")
        s_sb = work.tile([B, O, J], F32, name="s_sb")
        s2 = work.tile([B, O, J], F32, name="s2")
        sq = work.tile([B, O], F32, name="sq")
        t1 = work.tile([B, O], F32, name="t1")
        t2 = work.tile([B, O], F32, name="t2")
        den = work.tile([B, O], F32, name="den")
        rec = work.tile([B, O], F32, name="rec")
        wsc = work.tile([B, O], F32, name="wsc")
        v_sb = work.tile([B, O, J], F32, name="v_sb")
        # vz panels: b-partition staging for the block-diag v2 transposes.
        # vz0 cols 32q..32q+16 = v[b, 2q, :]; vz1 cols 32q+16..32q+32 =
        # v[b, 2q+1, :]; vzb0 cols 0:16 = v[b, 8, :]; vzb1 cols 16:32 =
        # v[b, 9, :]; all other columns stay zero forever.
        vz0 = work.tile([B, 128], BF16, name="vz0")
        vz1 = work.tile([B, 128], BF16, name="vz1")
        vzb0 = work.tile([B, 128], BF16, name="vzb0")
        vzb1 = work.tile([B, 128], BF16, name="vzb1")
        v2a = work.tile([128, 128], BF16, name="v2a")
        v2b = work.tile([128, 128], BF16, name="v2b")
        eps1 = work.tile([B, 1], F32, name="eps1")
        nc.vector.memset(eps1, EPS)
        for z in (vz0, vz1, vzb0, vzb1):
            nc.vector.memset(z, 0.0)

        # ------------- m1: s accumulation -------------
        def m1_it0():
            """s~ = sum_(i,d) W u for all 10 o at once (c=0.1 applied in
            squash).  Two alternating PSUM banks so consecutive matmuls
            pipeline past the PSUM access latency."""
            ps = psS.tile([B, O * J], F32, name="sps", tag="sps")
            for ch in range(NCHU):
                t, d = divmod(ch, D)
                nc.tensor.matmul(
                    ps, ui[:, t, d, :], ws[:, ch, :],
                    start=(ch == 0), stop=(ch == NCHU - 1),
                )
            return ps

        TBLK = ((0, 3), (3, 6), (6, 9))

        def m1_iter():
            """s_o = sum_(i,d) W_o (c_o*u), per-o cu stationary."""
            ps = psS.tile([B, O * J], F32, name="sps", tag="sps")
            for o in range(O):
                cu = cupool.tile([128, T, D, B], BF16, name="cu", tag="cu")
                if o in POOL_OS:
                    # GPSIMD stt is limited to 2D/3D APs: go per-t slice.
                    for t in range(T):
                        ebc = et[:, t, o, :].unsqueeze(1).broadcast_to(
                            [128, D, B])
                        nc.gpsimd.tensor_tensor(cu[:, t, :, :], ebc,
                                                uz[:, t, :, :], op=ALU.mult)
                else:
                    for t0, t1 in TBLK:
                        tb = slice(t0, t1)
                        ebc = et[:, tb, o, :].unsqueeze(2).broadcast_to(
                            [128, t1 - t0, D, B])
                        nc.vector.tensor_tensor(cu[:, tb, :, :], ebc,
                                                uz[:, tb, :, :], op=ALU.mult)
                for ch in range(NCHU):
                    t, d = divmod(ch, D)
                    nc.tensor.matmul(
                        ps[:, J * o : J * (o + 1)],
                        cu[:, t, d, :],
                        ws[:, ch, J * o : J * (o + 1)],
                        start=(ch == 0), stop=(ch == NCHU - 1),
                    )
            return ps

        # ------------- squash + v2 build -------------
        def squash(ps, scale, it):
            sf = s_sb.rearrange("b o j -> b (o j)")
            s2f = s2.rearrange("b o j -> b (o j)")
            nc.scalar.mul(sf, ps, scale)
            nc.vector.tensor_tensor(s2f, sf, sf, op=ALU.mult)
            nc.vector.tensor_reduce(sq, s2, axis=AX.X, op=ALU.add)
            nc.scalar.activation(t1, sq, ACTF.Ln, bias=eps1)
            nc.scalar.activation(t2, t1, ACTF.Exp, scale=0.5)  # sqrt(sq+eps)
            nc.vector.tensor_scalar_add(den, sq, 1.0)
            nc.vector.tensor_tensor(den, den, t2, op=ALU.mult)
            nc.vector.reciprocal_approx_accurate(rec, den, t1)
            nc.vector.tensor_tensor(wsc, sq, rec, op=ALU.mult)
            nc.vector.tensor_tensor(
                v_sb, s_sb, wsc.unsqueeze(2).broadcast_to([B, O, J]),
                op=ALU.mult)
            if it == 2:
                nc.sync.dma_start(out=vout_d[:, :, :], in_=v_sb)
                return
            vz0v = vz0.rearrange("b (q c) -> b q c", q=4)
            vz1v = vz1.rearrange("b (q c) -> b q c", q=4)
            nc.vector.tensor_copy(vz0v[:, :, 0:16], v_sb[:, 0:8:2, :])
            nc.vector.tensor_copy(vz1v[:, :, 16:32], v_sb[:, 1:9:2, :])
            nc.vector.tensor_copy(vzb0[:, 0:16], v_sb[:, 8, :])
            nc.vector.tensor_copy(vzb1[:, 16:32], v_sb[:, 9, :])
            # transpose the vz panels onto v2 via the PE (identity matmul)
            # instead of DMA transposes: no DMA queue contention and no
            # dge completion delay on the round-boundary critical path.
            pt = psT.tile([128, 256], F32, name="v2t", tag="v2t")
            nc.tensor.matmul(pt[:, 0:64], vz0, id64, start=True, stop=True)
            nc.tensor.matmul(pt[:, 64:128], vz1, id64, start=True, stop=True)
            nc.tensor.matmul(pt[0:32, 128:192], vzb0[:, 0:32], id64,
                             start=True, stop=True)
            nc.tensor.matmul(pt[0:32, 192:256], vzb1[:, 0:32], id64,
                             start=True, stop=True)
            nc.scalar.copy(v2a, pt[:, 0:128])
            nc.scalar.copy(v2b[0:32, :], pt[0:32, 128:256])

        # ------------- m2: agreement -> bl -------------
        def m2(it):
            for q in range(5):
                ug = ugpool.tile([128, KFLAT], BF16, name="ug", tag="ug")

                def mm(dst, nn):
                    csl = slice(512 * nn, 512 * (nn + 1))
                    if q < 4:
                        nc.tensor.matmul(
                            dst, v2a[32 * q : 32 * q + 32, :],
                            wba[32 * q : 32 * q + 32, csl],
                            start=True, stop=True,
                            tile_position=(32 * q, 0),
                        )
                    else:
                        nc.tensor.matmul(
                            dst, v2b[0:32, :], wbb[:, csl],
                            start=True, stop=True,
                            tile_position=(0, 0),
                        )

                def drain_pair(n1, n2):
                    pp = psM.tile([128, 1024], F32, name="m2p", tag="m2p")
                    mm(pp[:, 0:512], n1)
                    mm(pp[:, 512:1024], n2)
                    csl = slice(512 * n1, 512 * (n2 + 1))
                    nc.scalar.copy(ug[:, csl], pp)
                    if (n1, n2) in DVE_PAIRS:
                        nc.vector.tensor_tensor(ug[:, csl], ug[:, csl],
                                                ur[:, csl], op=ALU.mult)
                    else:
                        nc.gpsimd.tensor_tensor(ug[:, csl], ug[:, csl],
                                                ur[:, csl], op=ALU.mult)

                def direct(nn):
                    ps = psN.tile([128, 512], F32, name="m2d", tag="m2d")
                    mm(ps, nn)
                    csl = slice(512 * nn, 512 * (nn + 1))
                    nc.vector.tensor_tensor(ug[:, csl], ps, ur[:, csl],
                                            op=ALU.mult)

                for nn in DIRECT_NNS:
                    direct(nn)
                for n1, n2 in DRAIN_PAIRS:
                    drain_pair(n1, n2)
                # d-fold tree (d-major flat: level k folds d, d+4 / d+2 / d+1)
                # level 1 on DVE (bf16 2x rate), the f32-tainted tail on Pool.
                # The last pair (q=4) gates the next softmax, so its levels
                # are split DVE || Pool to cut the boundary latency.
                nc.vector.tensor_tensor(ug[:, 0:2048], ug[:, 0:2048],
                                        ug[:, 4608:6656], op=ALU.add)
                nc.vector.tensor_tensor(ug[:, 2048:4608], ug[:, 2048:4608],
                                        ug[:, 6656:9216], op=ALU.add)
                nc.vector.tensor_tensor(ug[:, 0:2304], ug[:, 0:2304],
                                        ug[:, 2304:4608], op=ALU.add)
                if it == 0:
                    nc.gpsimd.tensor_tensor(bl[:, q, :], ug[:, 0:I],
                                            ug[:, I : 2 * I], op=ALU.add)
                else:
                    tmp = ug[:, 2304 : 2304 + I]
                    nc.gpsimd.tensor_tensor(tmp, ug[:, 0:I],
                                            ug[:, I : 2 * I], op=ALU.add)
                    nc.gpsimd.tensor_tensor(bl[:, q, :], bl[:, q, :], tmp,
                                            op=ALU.add)

        # ------------- softmax (i-space) + u/Z fold -------------
        def softmax():
            # slot 4 (the last pair) is exp'd in t-thirds so the t-blocked
            # tail below can start before the whole slot is done.
            for p in range(4):
                nc.scalar.activation(e[:, p, :], bl[:, p, :], ACTF.Exp)
            for t0, t1 in TBLK:
                nc.scalar.activation(e[:, 4, 128 * t0 : 128 * t1],
                                     bl[:, 4, 128 * t0 : 128 * t1], ACTF.Exp)
            for o in range(O):
                sl, h = _oslot(o)
                for t in range(T):
                    nc.sync.dma_start_transpose(
                        out=et[:, t, o, :],
                        in_=e[64 * h : 64 * h + 64, sl,
                              128 * t : 128 * (t + 1)],
                    )
            # Per t-block: DVE sums o={0,2,4,6,8,9} (late pairs last), Pool
            # sums o={1,3,5,7}, DVE merges, recip, 1/Z fold into u.
            for t0, t1 in TBLK:
                tb = slice(t0, t1)
                nc.vector.tensor_tensor(zt[:, tb, :], et[:, tb, 0, :],
                                        et[:, tb, 2, :], op=ALU.add)
                for o in (4, 6, 8, 9):
                    nc.vector.tensor_tensor(zt[:, tb, :], zt[:, tb, :],
                                            et[:, tb, o, :], op=ALU.add)
                nc.gpsimd.tensor_tensor(ztp[:, tb, :], et[:, tb, 1, :],
                                        et[:, tb, 3, :], op=ALU.add)
                for o in (5, 7):
                    nc.gpsimd.tensor_tensor(ztp[:, tb, :], ztp[:, tb, :],
                                            et[:, tb, o, :], op=ALU.add)
                nc.vector.tensor_tensor(ztf[:, tb, :], zt[:, tb, :],
                                        ztp[:, tb, :], op=ALU.add)
                nc.vector.reciprocal_approx_fast(rz[:, tb, :], ztf[:, tb, :])
                nc.vector.tensor_copy(rzb[:, tb, :], rz[:, tb, :])
                if t0 == 3:
                    for t in range(t0, t1):
                        nc.gpsimd.tensor_tensor(
                            uz[:, t, :, :], ui[:, t, :, :],
                            rzb[:, t, :].unsqueeze(1).broadcast_to(
                                [128, D, B]),
                            op=ALU.mult)
                else:
                    nc.vector.tensor_tensor(
                        uz[:, tb, :, :], ui[:, tb, :, :],
                        rzb[:, tb, :].unsqueeze(2).broadcast_to(
                            [128, t1 - t0, D, B]),
                        op=ALU.mult)

        # ========================= flow =========================
        ps0 = m1_it0()
        squash(ps0, 0.1, 0)
        m2(0)
        softmax()
        ps1 = m1_iter()
        squash(ps1, 1.0, 1)
        m2(1)
        softmax()
        ps2 = m1_iter()
        squash(ps2, 1.0, 2)

    nc.finalize()
    return nc


def _host_prep(u, weights):
    """Per-core input maps. u [512,1152,8] f32, weights [1152,10,16,8] f32."""
    W = np.asarray(weights, dtype=np.float32)
    u = np.asarray(u, dtype=np.float32)
    # W_s[p, t*8+d, o*16+j] = W[t*128+p, o, j, d]
    ws = np.ascontiguousarray(
        W.reshape(T, 128, O, J, D).transpose(1, 0, 4, 2, 3)
    ).reshape(128, NCHU, O * J).astype(bfnp)
    # wb rows 32q+16h+j = W[o=2q+h][j, d-major flat]
    wt = W.transpose(1, 2, 3, 0).reshape(O, J, KFLAT)  # [o, j, (d,i)]
    wba = np.zeros((128, KFLAT), dtype=bfnp)
    for o in range(8):
        q, h = o // 2, o % 2
        wba[32 * q + 16 * h : 32 * q + 16 * h + 16, :] = wt[o].astype(bfnp)
    wbb = np.zeros((32, KFLAT), dtype=bfnp)
    for o in (8, 9):
        h = o - 8
        wbb[16 * h : 16 * h + 16, :] = wt[o].astype(bfnp)

    base = {"ws": ws, "wba": wba, "wbb": wbb,
            "id64": np.eye(B, dtype=bfnp)}
    in_maps = []
    for c in range(N_CORES):
        uc = u[c * B : (c + 1) * B]  # [64, 1152, 8]
        ui = np.ascontiguousarray(
            uc.reshape(B, T, 128, D).transpose(2, 1, 3, 0)
        ).astype(bfnp)  # [128, T, D, B]
        urh = np.ascontiguousarray(uc.transpose(0, 2, 1)).reshape(B, KFLAT)
        ur = np.concatenate([urh, urh], axis=0).astype(bfnp)  # [128, KFLAT]
        in_maps.append({**base, "ui": ui, "ur": ur})
    return in_maps


def kernel(u, weights):
    if "nc" not in _cache:
        _cache["nc"] = build_nc()
    nc = _cache["nc"]
    in_maps = _host_prep(u, weights)
    res = run_bass_kernel_spmd(nc, in_maps, core_ids=list(range(N_CORES)))
    out = np.concatenate([res.results[c]["vout"] for c in range(N_CORES)], axis=0)
    return out.astype(np.float32)


if __name__ == "__main__":
    rng = np.random.default_rng(0)
    u = rng.standard_normal((512, 1152, 8), dtype=np.float32)
    w = (rng.standard_normal((1152, 10, 16, 8)) * 0.1).astype(np.float32)
    v = kernel(u, w)
    print("out", v.shape, v.dtype, np.abs(v).max())
